# revision 10
# baseline (speedup 1.0000x reference)
"""DeepseekV2 MLA prefill attention on 8 NeuronCores (Trainium2, Bass/Tile).

Sharding: token-parallel attention with zigzag blocks (core c owns token
blocks {c, 15-c}); all large weights are uploaded row-sharded (1/8 per core)
and broadcast on-device via AllGather into internal DRAM, so every weight
byte crosses the host link exactly once. Weight tensors are pre-swizzled on
host so each SBUF slab load is partition-contiguous (KB-sized descriptors).

Attention softmax uses a constant exp bias (scores are bounded; verified
max |scale*s| ~ 5.5 << 16) so exp reads score PSUM directly -- no row-max
pass, no second exp pass. Normalization is folded into the P^T transpose by
multiplying against diag(1/rowsum) instead of the identity. P@V accumulates
across units in PSUM.

SPMD: one program for all cores; per-core variation carried by mask data.
"""
import sys
import json

sys.path.insert(0, "/opt/trn_rl_repo")

import numpy as np
import ml_dtypes

import concourse.bass as bass
import concourse.mybir as mybir
import concourse.tile as tile
from concourse.bass_utils import run_bass_kernel_spmd

F32 = mybir.dt.float32
F32R = mybir.dt.float32r
BF16 = mybir.dt.bfloat16

T = 2048
H = 32
HID = 5120
QL = 1536
KVL = 512
DN = 128
DR = 64
DQK = DN + DR
DV = 128
EPS = 1e-6
SCALING = DQK ** -0.5
NCORES = 8
OWN = 256
CH = 512
NEG = -1e30
EB = 16.0            # constant exp bias: exp(SCALING*s - EB)

HT = HID // 128      # 40
QLT = QL // 128      # 12
KVT = KVL // 128     # 4
KVT5 = KVT + 1       # 4 latent slabs + 1 (padded) rope slab

# attention units: u0-u2 = B-tile keys [0:512/512:1024/1024:1536],
# u3 = A-tile keys [0:512], u4 = A-tile keys [512:1024] (active c>=4),
# u5 = B-tile keys [1536:2048] (active c<4). Inactive units fully masked.
UQS = [1, 1, 1, 0, 0, 1]             # 1 = B tile (q cols 128:256)
UKO = [0, CH, 2 * CH, 0, CH, 3 * CH]
MASKED = [2, 3, 4, 5]                # units with an additive mask
AUNITS = (3, 4)
BUNITS = (0, 1, 2, 5)
# vh slot lists per unit (vfull rows are rank-major: slot k=2r+s ~ block
# b = r (s=0) or 15-r (s=1); token block b -> slot 2b if b<8 else 2(15-b)+1)
U_V = [[0, 2, 4, 6], [8, 10, 12, 14], [15, 13, 11, 9],
       [0, 2, 4, 6], [8, 10, 12, 14], [7, 5, 3, 1]]


def legalize_sync_waits(nc):
    """This container's walrus accepts at most one sync-wait per instruction;
    split extras onto standalone EventSemaphore waits just before (same
    engine; engine streams preserve intra-block order)."""
    m = json.loads(nc.to_json_bytes())
    ctr = [0]

    def fresh():
        ctr[0] += 1
        return f"I-lw-{ctr[0]}"

    for f in m["functions"]:
        for bb in f["blocks"]:
            out = []
            for ins in bb["instructions"]:
                si = ins.get("sync_info")
                waits = (si or {}).get("on_wait") or []
                if len(waits) > 1:
                    for w in waits[:-1]:
                        out.append({
                            "debug": ins.get("debug", 0),
                            "engine": ins["engine"],
                            "ins": [], "outs": [],
                            "name": fresh(),
                            "opcode": "EventSemaphore",
                            "sync_info": {"on_update": [], "on_wait": [w]},
                        })
                    si["on_wait"] = waits[-1:]
                out.append(ins)
            bb["instructions"] = out
    nc.m = mybir.module_from_json_bytes(json.dumps(m).encode())
    return nc


def build_bass():
    nc = bass.Bass()
    AL = mybir.AluOpType
    AF = mybir.ActivationFunctionType

    dp = nc.declare_dram_parameter
    hidp_d = dp("hidp", [128, HT * OWN], BF16, isOutput=False)
    cosT_d = dp("cosT", [DR, OWN], F32, isOutput=False)
    sinTs_d = dp("sinTs", [DR, OWN], F32, isOutput=False)
    wqa_d = dp("wqa_sh", [QL // 8, HID], BF16, isOutput=False)
    wkva_d = dp("wkva_sh", [KVT5 * 128 // 8, HID], BF16, isOutput=False)
    wqb_d = dp("wqb_sh", [H * 128 // 8, QLT * DQK], BF16, isOutput=False)
    wbv_d = dp("wbv_sh", [KVL // 8, H * DV], BF16, isOutput=False)
    wo_d = dp("wo_sh", [HID // 8, H * DV], BF16, isOutput=False)
    wbn_d = dp("wbn", [128, KVT * 4 * DN], BF16, isOutput=False)
    mask4_d = dp("mask4", [128, 4, CH], F32, isOutput=False)
    ident_d = dp("ident", [128, 128], BF16, isOutput=False)
    ones128_d = dp("ones128", [128, 1], F32R, isOutput=False)
    onesrow_d = dp("onesrow", [1, 128], F32, isOutput=False)
    outT_d = dp("outT", [HID, OWN], F32, isOutput=True)

    RG = [list(range(NCORES))]

    with tile.TileContext(nc) as tc:
        from contextlib import ExitStack
        st = ExitStack()
        const = st.enter_context(tc.tile_pool(name="const", bufs=1))
        dram = st.enter_context(tc.tile_pool(name="dram", bufs=1, space="DRAM"))

        # ---- AG destinations (internal DRAM, Shared) ----
        gwqa = dram.tile([QL, HID], BF16, addr_space="Shared")
        gwkva = dram.tile([KVT5 * 128, HID], BF16, addr_space="Shared")
        gwqb = dram.tile([H * 128, QLT * DQK], BF16, addr_space="Shared")
        gwbv = dram.tile([KVL, H * DV], BF16, addr_space="Shared")
        gwo = dram.tile([HID, H * DV], BF16, addr_space="Shared")
        agin = dram.tile([KVL + DR, OWN], BF16)
        agkv = dram.tile([NCORES * (KVL + DR), OWN], BF16, addr_space="Shared")
        vshard = dram.tile([OWN, H * DV], BF16)
        vfull = dram.tile([T, H * DV], BF16, addr_space="Shared")
        ktshard = dram.tile([4 * DN, T], BF16)
        ktfull = dram.tile([H * DN, T], BF16, addr_space="Shared")

        def ag(inp, outp):
            nc.gpsimd.collective_compute(
                "AllGather", AL.bypass, replica_groups=RG,
                ins=[inp.opt()], outs=[outp.opt()])

        # weight broadcasts: stage each shard param into internal DRAM
        # (collectives cannot read IO tensors), then AllGather. Ordered by
        # first use so the CC queue never blocks a consumer longer than
        # needed.
        swqa = dram.tile([QL // 8, HID], BF16)
        swkva = dram.tile([KVT5 * 128 // 8, HID], BF16)
        swqb = dram.tile([H * 128 // 8, QLT * DQK], BF16)
        swbv = dram.tile([KVL // 8, H * DV], BF16)
        swo = dram.tile([HID // 8, H * DV], BF16)
        nc.sync.dma_start(swqa[:], wqa_d[:])
        nc.sync.dma_start(swkva[:], wkva_d[:])
        nc.sync.dma_start(swbv[:], wbv_d[:])
        nc.sync.dma_start(swqb[:], wqb_d[:])
        nc.sync.dma_start(swo[:], wo_d[:])
        ag(swqa[:], gwqa[:])
        ag(swkva[:], gwkva[:])
        ag(swbv[:], gwbv[:])
        ag(swqb[:], gwqb[:])

        # ---- constants ----
        ident = const.tile([128, 128], BF16)
        nc.sync.dma_start(ident[:], ident_d[:])
        ones128 = const.tile([128, 1], F32R)
        nc.sync.dma_start(ones128[:], ones128_d[:])
        onesrow = const.tile([1, 128], F32)
        nc.sync.dma_start(onesrow[:], onesrow_d[:])
        mask4 = const.tile([128, 4, CH], F32)
        nc.sync.dma_start(mask4[:], mask4_d[:])
        cosT = const.tile([128, OWN], F32)
        nc.sync.dma_start(cosT[0:DR, :], cosT_d[:])
        nc.sync.dma_start(cosT[64:64 + DR, :], cosT_d[:])
        sinTs = const.tile([128, OWN], F32)
        nc.sync.dma_start(sinTs[0:DR, :], sinTs_d[:])
        nc.sync.dma_start(sinTs[64:64 + DR, :], sinTs_d[:])
        epsc = const.tile([1, 1], F32)
        nc.vector.memset(epsc[:], EPS)
        ebias = const.tile([128, 1], F32)
        nc.vector.memset(ebias[:], -EB)

        # =========== phase B: down projections (transposed) ===========
        latp = st.enter_context(tc.tile_pool(name="latp", bufs=1))
        ph = ExitStack()
        hidp = ph.enter_context(tc.tile_pool(name="hidp", bufs=1))
        wsl = ph.enter_context(tc.tile_pool(name="wsl", bufs=2))
        rawp = ph.enter_context(tc.tile_pool(name="rawp", bufs=1))
        psB = ph.enter_context(tc.tile_pool(name="psB", bufs=4, space="PSUM"))
        psS = ph.enter_context(tc.tile_pool(name="psS", bufs=2, space="PSUM"))

        hidT = hidp.tile([128, HT, OWN], BF16)
        nc.sync.dma_start(hidT[:], hidp_d.rearrange("p (a t) -> p a t", a=HT))

        latq = rawp.tile([128, QLT, OWN], F32)
        latkv = rawp.tile([128, KVT5, OWN], F32)

        for lt in range(QLT):
            wslab = wsl.tile([128, HT, 128], BF16, tag="wslab")
            nc.sync.dma_start(
                wslab[:], gwqa[128 * lt:128 * (lt + 1), :]
                .rearrange("p (a c) -> p a c", a=HT))
            ps = psB.tile([128, OWN], F32, tag="dps")
            for ht in range(HT):
                nc.tensor.matmul(ps[:], wslab[:, ht, :], hidT[:, ht, :],
                                 start=(ht == 0), stop=(ht == HT - 1))
            nc.scalar.copy(latq[:, lt, :], ps[:])

        for lt in range(KVT5):
            wslab = wsl.tile([128, HT, 128], BF16, tag="wslab")
            nc.sync.dma_start(
                wslab[:], gwkva[128 * lt:128 * (lt + 1), :]
                .rearrange("p (a c) -> p a c", a=HT))
            ps = psB.tile([128, OWN], F32, tag="dps")
            for ht in range(HT):
                nc.tensor.matmul(ps[:], wslab[:, ht, :], hidT[:, ht, :],
                                 start=(ht == 0), stop=(ht == HT - 1))
            nc.scalar.copy(latkv[:, lt, :], ps[:])

        # ---- rmsnorm factors via squares + ones-matmul ----
        latq_n = latp.tile([128, QLT, OWN], BF16)
        latkv_n = latp.tile([128, KVT, OWN], BF16)

        def rmsnorm(lat, lat_n, nt, L):
            ssq = psS.tile([1, OWN], F32, tag="ssq")
            for lt in range(nt):
                sq = rawp.tile([128, OWN], F32R, tag="sqscratch", bufs=2)
                nc.vector.tensor_tensor(out=sq[:], in0=lat[:, lt, :],
                                        in1=lat[:, lt, :], op=AL.mult)
                nc.tensor.matmul(ssq[:], ones128[:], sq[:],
                                 start=(lt == 0), stop=(lt == nt - 1))
            f = rawp.tile([1, OWN], F32, tag="fscratch", bufs=2)
            nc.scalar.activation(f[:], ssq[:], AF.Sqrt, bias=epsc[:],
                                 scale=1.0 / L)
            fr = rawp.tile([1, OWN], F32, tag="frscratch", bufs=2)
            nc.vector.reciprocal(fr[:], f[:])
            fb = psS.tile([128, OWN], F32, tag="fbcast")
            nc.tensor.matmul(fb[:], onesrow[:], fr[:], start=True, stop=True)
            for lt in range(nt):
                nc.vector.tensor_tensor(out=lat_n[:, lt, :], in0=lat[:, lt, :],
                                        in1=fb[:], op=AL.mult)

        rmsnorm(latq, latq_n, QLT, QL)
        rmsnorm(latkv, latkv_n, KVT, KVL)

        # ---- rope k_pe (deinterleave folded into wkva on host) ----
        kpsw = rawp.tile([128, OWN], F32)
        nc.sync.dma_start(kpsw[0:32, :], latkv[32:64, KVT, :])
        nc.sync.dma_start(kpsw[32:64, :], latkv[0:32, KVT, :])
        kpc = rawp.tile([128, OWN], F32)
        nc.vector.tensor_tensor(out=kpc[0:DR, :], in0=latkv[0:DR, KVT, :],
                                in1=cosT[0:DR, :], op=AL.mult)
        nc.vector.tensor_tensor(out=kpsw[0:DR, :], in0=kpsw[0:DR, :],
                                in1=sinTs[0:DR, :], op=AL.mult)
        kpeR = rawp.tile([128, OWN], BF16)
        nc.vector.tensor_tensor(out=kpeR[0:DR, :], in0=kpc[0:DR, :],
                                in1=kpsw[0:DR, :], op=AL.add)

        # assemble AG input: rows 0:512 normalized latent, 512:576 roped kpe
        for lt in range(KVT):
            nc.sync.dma_start(agin[128 * lt:128 * (lt + 1), :],
                              latkv_n[:, lt, :])
        nc.sync.dma_start(agin[KVL:KVL + DR, :], kpeR[0:DR, :])
        ag(agin[:], agkv[:])
        ph.close()

        # =========== phase D: V (own tokens, all heads) -> AG ===========
        ph = ExitStack()
        wv = ph.enter_context(tc.tile_pool(name="wv", bufs=2))
        psD = ph.enter_context(tc.tile_pool(name="psD", bufs=2, space="PSUM"))
        evp = ph.enter_context(tc.tile_pool(name="evp", bufs=3))

        for vc in range(8):             # 8 chunks of 512 v-columns
            wvs = wv.tile([128, KVT, CH], BF16, tag="wvs")
            nc.sync.dma_start(
                wvs[:], gwbv[:, CH * vc:CH * (vc + 1)]
                .rearrange("(l p) c -> p l c", p=128))
            for tt in range(2):         # 2 token tiles of 128
                ps = psD.tile([128, CH], F32, tag="vps")
                for lt in range(KVT):
                    nc.tensor.matmul(
                        ps[:], latkv_n[:, lt, 128 * tt:128 * (tt + 1)],
                        wvs[:, lt, :], start=(lt == 0), stop=(lt == KVT - 1))
                ev = evp.tile([128, CH], BF16, tag="vev")
                nc.scalar.copy(ev[:], ps[:])
                nc.sync.dma_start(
                    vshard[128 * tt:128 * (tt + 1), CH * vc:CH * (vc + 1)],
                    ev[:])

        ag(vshard[:], vfull[:])

        # =========== phase E: K^T (this core's 4 heads, all tokens) -> AG ====
        wkn = ph.enter_context(tc.tile_pool(name="wkn", bufs=1))
        wkns = wkn.tile([128, KVT, 4 * DN], BF16)
        nc.sync.dma_start(wkns[:], wbn_d.rearrange("p (l c) -> p l c", l=KVT))

        agp = ph.enter_context(tc.tile_pool(name="agp", bufs=2))
        for r in range(NCORES):
            slab = agp.tile([128, KVT, OWN], BF16, tag="agslab")
            nc.sync.dma_start(
                slab[:], agkv[(KVL + DR) * r:(KVL + DR) * r + KVL, :]
                .rearrange("(l p) t -> p l t", p=128))
            for hl in range(4):
                ps = psD.tile([128, OWN], F32, tag="ktps")
                for lt in range(KVT):
                    nc.tensor.matmul(ps[:], wkns[:, lt, DN * hl:DN * (hl + 1)],
                                     slab[:, lt, :],
                                     start=(lt == 0), stop=(lt == KVT - 1))
                ev = evp.tile([128, OWN], BF16, tag="ktev")
                nc.scalar.copy(ev[:], ps[:])
                # token-ordered columns: chunk r covers blocks r and 15-r
                nc.sync.dma_start(
                    ktshard[DN * hl:DN * (hl + 1), 128 * r:128 * (r + 1)],
                    ev[:, 0:128])
                nc.sync.dma_start(
                    ktshard[DN * hl:DN * (hl + 1),
                            128 * (15 - r):128 * (16 - r)],
                    ev[:, 128:256])

        ag(ktshard[:], ktfull[:])
        ag(swo[:], gwo[:])

        # k_pe^T assembly (token-ordered, shared across heads)
        kpeT = const.tile([128, T], BF16)
        for b in range(16):
            rb = min(b, 15 - b)
            colsl = slice(0, 128) if b < 8 else slice(128, 256)
            src_ap = agkv[(KVL + DR) * rb + KVL:(KVL + DR) * rb + KVL + DR,
                          colsl]
            nc.sync.dma_start(kpeT[0:DR, 128 * b:128 * (b + 1)], src_ap)
            nc.sync.dma_start(kpeT[64:64 + DR, 128 * b:128 * (b + 1)], src_ap)
        ph.close()

        # =========== phase F: Q up-projection + rope (all heads) ===========
        qp_pool = st.enter_context(tc.tile_pool(name="qp", bufs=1))
        qTn = qp_pool.tile([128, H, OWN], BF16)
        qTp = qp_pool.tile([128, H // 2, OWN], BF16)
        attnT = qp_pool.tile([128, H, OWN], F32R)

        ph = ExitStack()
        wqb = ph.enter_context(tc.tile_pool(name="wqb", bufs=2))
        psF = ph.enter_context(tc.tile_pool(name="psF", bufs=3, space="PSUM"))
        rp = ph.enter_context(tc.tile_pool(name="rp", bufs=3))

        for h in range(H):
            ws = wqb.tile([128, QLT, DQK], BF16, tag="wqbs")
            nc.sync.dma_start(
                ws[:], gwqb[128 * h:128 * (h + 1), :]
                .rearrange("p (l c) -> p l c", l=QLT))
            pb = 0 if h < 16 else 64
            hs_ = h % 16
            psn = psF.tile([128, OWN], F32, tag="qnps")
            psp = psF.tile([128, OWN], F32, tag="qpps")
            for lt in range(QLT):
                nc.tensor.matmul(psn[:], ws[:, lt, 0:DN], latq_n[:, lt, :],
                                 start=(lt == 0), stop=(lt == QLT - 1))
            for lt in range(QLT):
                nc.tensor.matmul(psp[0:DR, :], ws[:, lt, DN:DQK],
                                 latq_n[:, lt, :],
                                 start=(lt == 0), stop=(lt == QLT - 1))
            nc.scalar.copy(qTn[:, h, :], psn[:])
            praw = rp.tile([128, OWN], F32, tag="praw")
            nc.scalar.copy(praw[0:DR, :], psp[0:DR, :])
            psw = rp.tile([128, OWN], F32, tag="psw")
            nc.sync.dma_start(psw[0:32, :], praw[32:DR, :])
            nc.sync.dma_start(psw[32:DR, :], praw[0:32, :])
            pc = rp.tile([128, OWN], F32, tag="pc")
            nc.vector.tensor_tensor(out=pc[0:DR, :], in0=praw[0:DR, :],
                                    in1=cosT[0:DR, :], op=AL.mult)
            nc.vector.tensor_tensor(out=psw[0:DR, :], in0=psw[0:DR, :],
                                    in1=sinTs[0:DR, :], op=AL.mult)
            if pb == 0:
                nc.vector.tensor_tensor(out=qTp[0:DR, hs_, :], in0=pc[0:DR, :],
                                        in1=psw[0:DR, :], op=AL.add)
            else:
                rshift = rp.tile([128, OWN], BF16, tag="rshift")
                nc.vector.tensor_tensor(out=rshift[0:DR, :], in0=pc[0:DR, :],
                                        in1=psw[0:DR, :], op=AL.add)
                nc.sync.dma_start(qTp[pb:pb + DR, hs_, :], rshift[0:DR, :])
        ph.close()

        # =========== phase G: attention ===========
        ph = ExitStack()
        ap = ph.enter_context(tc.tile_pool(name="ap", bufs=2))
        sp_pool = ph.enter_context(tc.tile_pool(name="spp", bufs=2))
        stt = ph.enter_context(tc.tile_pool(name="stt", bufs=2))
        psG = ph.enter_context(tc.tile_pool(name="psG", bufs=3, space="PSUM"))
        psT = ph.enter_context(tc.tile_pool(name="psT", bufs=2, space="PSUM"))
        psV = ph.enter_context(tc.tile_pool(name="psV", bufs=3, space="PSUM"))

        for h in range(H):
            kt = ap.tile([128, T], BF16, tag="kt")
            nc.sync.dma_start(kt[:], ktfull[DN * h:DN * (h + 1), :])
            vh = ap.tile([128, 16, DV], BF16, tag="vh")
            nc.sync.dma_start(
                vh[:], vfull.rearrange("(k p) c -> p k c", p=128)
                [:, :, DV * h:DV * (h + 1)])

            pb = 0 if h < 16 else 64
            hs_ = h % 16

            # scores per unit: nope+rope MMs, mask (in PSUM), exp -> escr + sum
            Pb = sp_pool.tile([128, 6, CH], BF16, tag="Pb")
            sumu = stt.tile([128, 6], F32, tag="sumu")
            escrs = []
            for u in (0, 1, 2, 5, 3, 4):
                q0 = 128 * UQS[u]
                ps = psG.tile([128, CH], F32, tag="sps", name=f"s{h}_{u}")
                nc.tensor.matmul(ps[:], qTn[:, h, q0:q0 + 128],
                                 kt[:, UKO[u]:UKO[u] + CH],
                                 start=True, stop=False)
                nc.tensor.matmul(ps[:], qTp[pb:pb + DR, hs_, q0:q0 + 128],
                                 kpeT[pb:pb + DR, UKO[u]:UKO[u] + CH],
                                 start=False, stop=True)
                if u in MASKED:
                    mi = MASKED.index(u)
                    nc.vector.tensor_tensor(out=ps[:], in0=ps[:],
                                            in1=mask4[:, mi, :], op=AL.add)
                escr = sp_pool.tile([128, CH], F32, tag="escr", bufs=6,
                                    name=f"e{h}_{u}")
                nc.scalar.activation(escr[:], ps[:], AF.Exp,
                                     bias=ebias[:], scale=SCALING,
                                     accum_out=sumu[:, u:u + 1])
                escrs.append(escr)

            # group sums -> diag(1/sum) matrices
            sB0 = stt.tile([128, 1], F32, tag="sB0")
            nc.vector.tensor_tensor(out=sB0[:], in0=sumu[:, 0:1],
                                    in1=sumu[:, 1:2], op=AL.add)
            sB1 = stt.tile([128, 1], F32, tag="sB1")
            nc.vector.tensor_tensor(out=sB1[:], in0=sumu[:, 2:3],
                                    in1=sumu[:, 5:6], op=AL.add)
            sB = stt.tile([128, 1], F32, tag="sB")
            nc.vector.tensor_tensor(out=sB[:], in0=sB0[:], in1=sB1[:],
                                    op=AL.add)
            sA = stt.tile([128, 1], F32, tag="sA")
            nc.vector.tensor_tensor(out=sA[:], in0=sumu[:, 3:4],
                                    in1=sumu[:, 4:5], op=AL.add)
            rA = stt.tile([128, 1], F32, tag="rA")
            nc.vector.reciprocal(rA[:], sA[:])
            rB = stt.tile([128, 1], F32, tag="rB")
            nc.vector.reciprocal(rB[:], sB[:])

            # normalize during the f32->bf16 cast (q is on partitions here)
            for i, u in enumerate((0, 1, 2, 5, 3, 4)):
                r_ = rA if u in AUNITS else rB
                nc.gpsimd.tensor_scalar_mul(Pb[:, u, :], escrs[i][:], r_[:])

            # P^T + PV accumulated in PSUM
            psVB = psV.tile([128, DV], F32, tag="pv", name=f"pvB{h}")
            psVA = psV.tile([128, DV], F32, tag="pv", name=f"pvA{h}")
            nmm = 0
            for u in range(6):
                isA = u in AUNITS
                dst = psVA if isA else psVB
                first = (u == 3 and True) if isA else (u == 0)
                for kb in range(4):
                    tp = psT.tile([128, 128], BF16, tag="tp")
                    nc.tensor.transpose(tp[:], Pb[:, u, 128 * kb:128 * (kb + 1)],
                                        ident[:])
                    ptT = stt.tile([128, 128], BF16, tag="ptT", bufs=4)
                    if nmm % 2 == 0:
                        nc.vector.tensor_copy(ptT[:], tp[:])
                    else:
                        nc.scalar.copy(ptT[:], tp[:])
                    nmm += 1
                    nc.tensor.matmul(
                        dst[:], vh[:, U_V[u][kb], :], ptT[:],
                        start=(first and kb == 0),
                        stop=((u == 4 and kb == 3) if isA
                              else (u == 5 and kb == 3)))
            nc.scalar.copy(attnT[:, h, 0:128], psVA[:])
            nc.vector.tensor_copy(attnT[:, h, 128:256], psVB[:])
        ph.close()

        # =========== phase H: out projection ===========
        ph = ExitStack()
        wop = ph.enter_context(tc.tile_pool(name="wop", bufs=2))
        psH = ph.enter_context(tc.tile_pool(name="psH", bufs=4, space="PSUM"))
        oev = ph.enter_context(tc.tile_pool(name="oev", bufs=3))
        abf_p = ph.enter_context(tc.tile_pool(name="abf", bufs=1))
        attnB = abf_p.tile([128, H, OWN], BF16)
        for ct in range(H):
            nc.vector.tensor_copy(attnB[:, ct, :], attnT[:, ct, :])
        for oc in range(HID // 128):
            ws = wop.tile([128, H, 128], BF16, tag="wos")
            nc.sync.dma_start(
                ws[:], gwo[128 * oc:128 * (oc + 1), :]
                .rearrange("p (t c) -> p t c", t=H))
            ps = psH.tile([128, OWN], F32, tag="ops")
            for ct in range(H):
                nc.tensor.matmul(ps[:], ws[:, ct, :], attnB[:, ct, :],
                                 start=(ct == 0), stop=(ct == H - 1))
            ev = oev.tile([128, OWN], F32, tag="oev")
            nc.scalar.copy(ev[:], ps[:])
            nc.sync.dma_start(outT_d[128 * oc:128 * (oc + 1), :], ev[:])
        ph.close()
        st.close()

    nc.finalize()
    legalize_sync_waits(nc)
    return nc


_DEINT = np.array([2 * r if r < 32 else 2 * r - 63 for r in range(DR)])


def _host_prep(inputs):
    f32 = np.float32
    bf16 = ml_dtypes.bfloat16
    hs = np.asarray(inputs["hidden_states"], f32)
    cos = np.asarray(inputs["cos"], f32).reshape(T, DR)
    sin = np.asarray(inputs["sin"], f32).reshape(T, DR)
    wq_a = np.asarray(inputs["wq_a"], f32)
    q_ln = np.asarray(inputs["q_a_ln_w"], f32)
    wq_b = np.asarray(inputs["wq_b"], f32)
    wkv_a = np.asarray(inputs["wkv_a"], f32)
    kv_ln = np.asarray(inputs["kv_a_ln_w"], f32)
    wkv_b = np.asarray(inputs["wkv_b"], f32)
    wo = np.asarray(inputs["wo"], f32)

    # fold ln weights into up-projections
    wq_b = wq_b * q_ln[:, None]
    wkv_b = wkv_b * kv_ln[:, None]

    # deinterleave fold: q_pe columns of wq_b, k_pe columns of wkv_a
    wqbp = wq_b.copy()
    for h in range(H):
        pe = wq_b[:, h * DQK + DN:h * DQK + DQK]
        wqbp[:, h * DQK + DN:h * DQK + DQK] = pe[:, _DEINT]
    wkvap = np.zeros((HID, KVT5 * 128), f32)
    wkvap[:, :KVL] = wkv_a[:, :KVL]
    wkvap[:, KVL:KVL + DR] = wkv_a[:, KVL:][:, _DEINT]

    # split wkv_b into nope / v column groups (head-major)
    wkvbn = np.concatenate(
        [wkv_b[:, h * 256:h * 256 + DN] for h in range(H)], axis=1)
    wkvbv = np.concatenate(
        [wkv_b[:, h * 256 + DN:h * 256 + 256] for h in range(H)], axis=1)

    # swizzles: slab-major, partition-contiguous layouts
    WQA = wq_a.reshape(HT, 128, QLT, 128).transpose(2, 1, 0, 3) \
        .reshape(QL, HID).astype(bf16)
    WKVA = wkvap.reshape(HT, 128, KVT5, 128).transpose(2, 1, 0, 3) \
        .reshape(KVT5 * 128, HID).astype(bf16)
    WQB = wqbp.reshape(QLT, 128, H, DQK).transpose(2, 1, 0, 3) \
        .reshape(H * 128, QLT * DQK).astype(bf16)
    WBV = wkvbv.astype(bf16)                      # [512, 4096] natural
    WO = wo.reshape(H, 128, HT, 128).transpose(2, 1, 0, 3) \
        .reshape(HID, H * DV).astype(bf16)

    cosT = np.ascontiguousarray(cos.T)
    sinT = np.ascontiguousarray(sin.T)
    sinTs = sinT.copy()
    sinTs[0:32] = -sinT[0:32]

    ident = np.eye(128, dtype=bf16)
    ones128 = np.ones((128, 1), f32)
    onesrow = np.ones((1, 128), f32)

    def shard(a):
        n = a.shape[0] // NCORES
        return [np.ascontiguousarray(a[c * n:(c + 1) * n]) for c in
                range(NCORES)]

    WQA_s, WKVA_s, WQB_s, WBV_s, WO_s = (shard(WQA), shard(WKVA), shard(WQB),
                                         shard(WBV), shard(WO))

    qr = np.arange(128)[:, None]
    kr = np.arange(CH)[None, :]

    in_maps = []
    for c in range(NCORES):
        bA, bB = c, 15 - c
        own = np.r_[np.arange(128 * bA, 128 * bA + 128),
                    np.arange(128 * bB, 128 * bB + 128)]
        # masks for units u2,u3,u4,u5
        mask4 = np.zeros((128, 4, CH), f32)
        specs = [(bB, 2 * CH, True),          # u2
                 (bA, 0, True),               # u3
                 (bA, CH, c >= 4),            # u4
                 (bB, 3 * CH, c < 4)]         # u5
        for mi, (qb, koff, active) in enumerate(specs):
            if not active:
                mask4[:, mi, :] = NEG
            else:
                qtok = 128 * qb + qr
                ktok = koff + kr
                mask4[:, mi, :] = np.where(ktok <= qtok, 0.0, NEG)

        hid_own = np.ascontiguousarray(hs[own].T)      # [5120, 256]
        hidp = hid_own.reshape(HT, 128, OWN).transpose(1, 0, 2) \
            .reshape(128, HT * OWN).astype(bf16)

        wbn_c = wkvbn[:, 4 * DN * c:4 * DN * (c + 1)]  # [512, 512]
        wbn = wbn_c.reshape(KVT, 128, 4 * DN).transpose(1, 0, 2) \
            .reshape(128, KVT * 4 * DN).astype(bf16)

        in_maps.append({
            "hidp": hidp,
            "cosT": np.ascontiguousarray(cosT[:, own]),
            "sinTs": np.ascontiguousarray(sinTs[:, own]),
            "wqa_sh": WQA_s[c],
            "wkva_sh": WKVA_s[c],
            "wqb_sh": WQB_s[c],
            "wbv_sh": WBV_s[c],
            "wo_sh": WO_s[c],
            "wbn": wbn,
            "mask4": mask4,
            "ident": ident, "ones128": ones128, "onesrow": onesrow,
        })
    return in_maps


_NC_CACHE = None


def _get_nc():
    global _NC_CACHE
    if _NC_CACHE is None:
        _NC_CACHE = build_bass()
    return _NC_CACHE


def run(inputs, trace=False):
    nc = _get_nc()
    in_maps = _host_prep(inputs)
    res = run_bass_kernel_spmd(nc, in_maps, list(range(NCORES)), trace=trace)
    out = np.empty((T, HID), np.float32)
    for c in range(NCORES):
        oT = res.results[c]["outT"]
        out[128 * c:128 * (c + 1)] = oT[:, 0:128].T
        out[128 * (15 - c):128 * (16 - c)] = oT[:, 128:256].T
    return out, res


def kernel(**inputs):
    out, _ = run(inputs, trace=False)
    return out


# revision 12
# speedup vs baseline: 1.7451x; 1.7451x over previous
"""DeepseekV2 MLA prefill attention on 8 NeuronCores (Trainium2, Bass/Tile).

Sharding: token-parallel attention with zigzag blocks (core c owns token
blocks {c, 15-c}); all large weights are uploaded row-sharded (1/8 per core)
and broadcast on-device via AllGather into internal DRAM, so every weight
byte crosses the host link exactly once. Weight tensors are pre-swizzled on
host so each SBUF slab load is partition-contiguous (KB-sized descriptors).

Attention softmax uses a constant exp bias (scores are bounded; verified
max |scale*s| ~ 5.5 << 16) so exp reads score PSUM directly -- no row-max
pass, no second exp pass. Normalization is folded into the P^T transpose by
multiplying against diag(1/rowsum) instead of the identity. P@V accumulates
across units in PSUM.

SPMD: one program for all cores; per-core variation carried by mask data.
"""
import sys
import json

sys.path.insert(0, "/opt/trn_rl_repo")

import numpy as np
import ml_dtypes

import concourse.bass as bass
import concourse.mybir as mybir
import concourse.tile as tile
from concourse.bass_utils import run_bass_kernel_spmd

F32 = mybir.dt.float32
F32R = mybir.dt.float32r
BF16 = mybir.dt.bfloat16

T = 2048
H = 32
HID = 5120
QL = 1536
KVL = 512
DN = 128
DR = 64
DQK = DN + DR
DV = 128
EPS = 1e-6
SCALING = DQK ** -0.5
NCORES = 8
OWN = 256
CH = 512
NEG = -1e30
EB = 16.0            # constant exp bias: exp(SCALING*s - EB)

HT = HID // 128      # 40
QLT = QL // 128      # 12
KVT = KVL // 128     # 4
KVT5 = KVT + 1       # 4 latent slabs + 1 (padded) rope slab

# attention units: u0-u2 = B-tile keys [0:512/512:1024/1024:1536],
# u3 = A-tile keys [0:512], u4 = A-tile keys [512:1024] (active c>=4),
# u5 = B-tile keys [1536:2048] (active c<4). Inactive units fully masked.
UQS = [1, 1, 1, 0, 0, 1]             # 1 = B tile (q cols 128:256)
UKO = [0, CH, 2 * CH, 0, CH, 3 * CH]
MASKED = [2, 3, 4, 5]                # units with an additive mask
AUNITS = (3, 4)
BUNITS = (0, 1, 2, 5)
# vh slot lists per unit (vfull rows are rank-major: slot k=2r+s ~ block
# b = r (s=0) or 15-r (s=1); token block b -> slot 2b if b<8 else 2(15-b)+1)
U_V = [[0, 2, 4, 6], [8, 10, 12, 14], [15, 13, 11, 9],
       [0, 2, 4, 6], [8, 10, 12, 14], [7, 5, 3, 1]]


def legalize_sync_waits(nc):
    """This container's walrus accepts at most one sync-wait per instruction;
    split extras onto standalone EventSemaphore waits just before (same
    engine; engine streams preserve intra-block order)."""
    m = json.loads(nc.to_json_bytes())
    ctr = [0]

    def fresh():
        ctr[0] += 1
        return f"I-lw-{ctr[0]}"

    for f in m["functions"]:
        for bb in f["blocks"]:
            out = []
            for ins in bb["instructions"]:
                si = ins.get("sync_info")
                waits = (si or {}).get("on_wait") or []
                if len(waits) > 1:
                    for w in waits[:-1]:
                        out.append({
                            "debug": ins.get("debug", 0),
                            "engine": ins["engine"],
                            "ins": [], "outs": [],
                            "name": fresh(),
                            "opcode": "EventSemaphore",
                            "sync_info": {"on_update": [], "on_wait": [w]},
                        })
                    si["on_wait"] = waits[-1:]
                out.append(ins)
            bb["instructions"] = out
    nc.m = mybir.module_from_json_bytes(json.dumps(m).encode())
    return nc


def build_bass():
    nc = bass.Bass()
    AL = mybir.AluOpType
    AF = mybir.ActivationFunctionType

    dp = nc.declare_dram_parameter
    hidp_d = dp("hidp", [128, HT * OWN], BF16, isOutput=False)
    cosT_d = dp("cosT", [DR, OWN], F32, isOutput=False)
    sinTs_d = dp("sinTs", [DR, OWN], F32, isOutput=False)
    wqa_d = dp("wqa_sh", [QL // 8, HID], BF16, isOutput=False)
    wkva_d = dp("wkva_sh", [KVT5 * 128 // 8, HID], BF16, isOutput=False)
    wqb_d = dp("wqb_sh", [H * 128 // 8, QLT * DQK], BF16, isOutput=False)
    wbv_d = dp("wbv_sh", [KVL // 8, H * DV], BF16, isOutput=False)
    wo_d = dp("wo_sh", [HID // 8, H * DV], BF16, isOutput=False)
    wbn_d = dp("wbn", [128, KVT * 4 * DN], BF16, isOutput=False)
    mask4_d = dp("mask4", [128, 4, CH], F32, isOutput=False)
    ident_d = dp("ident", [128, 128], BF16, isOutput=False)
    ones128_d = dp("ones128", [128, 1], F32R, isOutput=False)
    onesrow_d = dp("onesrow", [1, 128], F32, isOutput=False)
    outT_d = dp("outT", [HID, OWN], F32, isOutput=True)

    RG = [list(range(NCORES))]

    with tile.TileContext(nc) as tc:
        from contextlib import ExitStack
        st = ExitStack()
        const = st.enter_context(tc.tile_pool(name="const", bufs=1))
        dram = st.enter_context(tc.tile_pool(name="dram", bufs=1, space="DRAM"))

        # ---- AG destinations (internal DRAM, Shared) ----
        gwqa = dram.tile([QL, HID], BF16, addr_space="Shared")
        gwkva = dram.tile([KVT5 * 128, HID], BF16, addr_space="Shared")
        gwqb = dram.tile([H * 128, QLT * DQK], BF16, addr_space="Shared")
        gwbv = dram.tile([KVL, H * DV], BF16, addr_space="Shared")
        gwo = dram.tile([HID, H * DV], BF16, addr_space="Shared")
        agin = dram.tile([KVL + DR, OWN], BF16)
        agkv = dram.tile([NCORES * (KVL + DR), OWN], BF16, addr_space="Shared")
        vshard = dram.tile([OWN, H * DV], BF16)
        vfull = dram.tile([T, H * DV], BF16, addr_space="Shared")
        ktshard = dram.tile([4 * DN, T], BF16)
        ktfull = dram.tile([H * DN, T], BF16, addr_space="Shared")

        def ag(inp, outp):
            nc.gpsimd.collective_compute(
                "AllGather", AL.bypass, replica_groups=RG,
                ins=[inp.opt()], outs=[outp.opt()])

        # weight broadcasts: stage each shard param into internal DRAM
        # (collectives cannot read IO tensors), then AllGather. Ordered by
        # first use so the CC queue never blocks a consumer longer than
        # needed.
        swqa = dram.tile([QL // 8, HID], BF16)
        swkva = dram.tile([KVT5 * 128 // 8, HID], BF16)
        swqb = dram.tile([H * 128 // 8, QLT * DQK], BF16)
        swbv = dram.tile([KVL // 8, H * DV], BF16)
        swo = dram.tile([HID // 8, H * DV], BF16)
        nc.sync.dma_start(swqa[:], wqa_d[:])
        nc.sync.dma_start(swkva[:], wkva_d[:])
        nc.sync.dma_start(swbv[:], wbv_d[:])
        nc.sync.dma_start(swqb[:], wqb_d[:])
        ag(swqa[:], gwqa[:])
        ag(swkva[:], gwkva[:])
        ag(swbv[:], gwbv[:])
        ag(swqb[:], gwqb[:])

        # ---- constants ----
        ident = const.tile([128, 128], BF16)
        nc.sync.dma_start(ident[:], ident_d[:])
        ones128 = const.tile([128, 1], F32R)
        nc.sync.dma_start(ones128[:], ones128_d[:])
        onesrow = const.tile([1, 128], F32)
        nc.sync.dma_start(onesrow[:], onesrow_d[:])
        mask4 = const.tile([128, 4, CH], F32)
        nc.sync.dma_start(mask4[:], mask4_d[:])
        cosT = const.tile([128, OWN], F32)
        nc.sync.dma_start(cosT[0:DR, :], cosT_d[:])
        nc.sync.dma_start(cosT[64:64 + DR, :], cosT_d[:])
        sinTs = const.tile([128, OWN], F32)
        nc.sync.dma_start(sinTs[0:DR, :], sinTs_d[:])
        nc.sync.dma_start(sinTs[64:64 + DR, :], sinTs_d[:])
        epsc = const.tile([1, 1], F32)
        nc.vector.memset(epsc[:], EPS)
        ebias = const.tile([128, 1], F32)
        nc.vector.memset(ebias[:], -EB)

        # =========== phase B: down projections (transposed) ===========
        latp = st.enter_context(tc.tile_pool(name="latp", bufs=1))
        ph = ExitStack()
        hidp = ph.enter_context(tc.tile_pool(name="hidp", bufs=1))
        wsl = ph.enter_context(tc.tile_pool(name="wsl", bufs=2))
        rawp = ph.enter_context(tc.tile_pool(name="rawp", bufs=1))
        psB = ph.enter_context(tc.tile_pool(name="psB", bufs=4, space="PSUM"))
        psS = ph.enter_context(tc.tile_pool(name="psS", bufs=2, space="PSUM"))

        hidT = hidp.tile([128, HT, OWN], BF16)
        nc.sync.dma_start(hidT[:], hidp_d.rearrange("p (a t) -> p a t", a=HT))

        latq = rawp.tile([128, QLT, OWN], F32)
        latkv = rawp.tile([128, KVT5, OWN], F32)

        for lt in range(QLT):
            wslab = wsl.tile([128, HT, 128], BF16, tag="wslab")
            nc.sync.dma_start(
                wslab[:], gwqa[128 * lt:128 * (lt + 1), :]
                .rearrange("p (a c) -> p a c", a=HT))
            ps = psB.tile([128, OWN], F32, tag="dps")
            for ht in range(HT):
                nc.tensor.matmul(ps[:], wslab[:, ht, :], hidT[:, ht, :],
                                 start=(ht == 0), stop=(ht == HT - 1))
            nc.scalar.copy(latq[:, lt, :], ps[:])

        for lt in range(KVT5):
            wslab = wsl.tile([128, HT, 128], BF16, tag="wslab")
            nc.sync.dma_start(
                wslab[:], gwkva[128 * lt:128 * (lt + 1), :]
                .rearrange("p (a c) -> p a c", a=HT))
            ps = psB.tile([128, OWN], F32, tag="dps")
            for ht in range(HT):
                nc.tensor.matmul(ps[:], wslab[:, ht, :], hidT[:, ht, :],
                                 start=(ht == 0), stop=(ht == HT - 1))
            nc.scalar.copy(latkv[:, lt, :], ps[:])

        # ---- rmsnorm factors via squares + ones-matmul ----
        latq_n = latp.tile([128, QLT, OWN], BF16)
        latkv_n = latp.tile([128, KVT, OWN], BF16)

        def rmsnorm(lat, lat_n, nt, L):
            ssq = psS.tile([1, OWN], F32, tag="ssq")
            for lt in range(nt):
                sq = rawp.tile([128, OWN], F32R, tag="sqscratch", bufs=2)
                nc.vector.tensor_tensor(out=sq[:], in0=lat[:, lt, :],
                                        in1=lat[:, lt, :], op=AL.mult)
                nc.tensor.matmul(ssq[:], ones128[:], sq[:],
                                 start=(lt == 0), stop=(lt == nt - 1))
            f = rawp.tile([1, OWN], F32, tag="fscratch", bufs=2)
            nc.scalar.activation(f[:], ssq[:], AF.Sqrt, bias=epsc[:],
                                 scale=1.0 / L)
            fr = rawp.tile([1, OWN], F32, tag="frscratch", bufs=2)
            nc.vector.reciprocal(fr[:], f[:])
            fb = psS.tile([128, OWN], F32, tag="fbcast")
            nc.tensor.matmul(fb[:], onesrow[:], fr[:], start=True, stop=True)
            for lt in range(nt):
                nc.vector.tensor_tensor(out=lat_n[:, lt, :], in0=lat[:, lt, :],
                                        in1=fb[:], op=AL.mult)

        rmsnorm(latq, latq_n, QLT, QL)
        rmsnorm(latkv, latkv_n, KVT, KVL)

        # ---- rope k_pe (deinterleave folded into wkva on host) ----
        kpsw = rawp.tile([128, OWN], F32)
        nc.sync.dma_start(kpsw[0:32, :], latkv[32:64, KVT, :])
        nc.sync.dma_start(kpsw[32:64, :], latkv[0:32, KVT, :])
        kpc = rawp.tile([128, OWN], F32)
        nc.vector.tensor_tensor(out=kpc[0:DR, :], in0=latkv[0:DR, KVT, :],
                                in1=cosT[0:DR, :], op=AL.mult)
        nc.vector.tensor_tensor(out=kpsw[0:DR, :], in0=kpsw[0:DR, :],
                                in1=sinTs[0:DR, :], op=AL.mult)
        kpeR = rawp.tile([128, OWN], BF16)
        nc.vector.tensor_tensor(out=kpeR[0:DR, :], in0=kpc[0:DR, :],
                                in1=kpsw[0:DR, :], op=AL.add)

        # assemble AG input: rows 0:512 normalized latent, 512:576 roped kpe
        for lt in range(KVT):
            nc.sync.dma_start(agin[128 * lt:128 * (lt + 1), :],
                              latkv_n[:, lt, :])
        nc.sync.dma_start(agin[KVL:KVL + DR, :], kpeR[0:DR, :])
        ag(agin[:], agkv[:])
        ph.close()

        # =========== phase D: V (own tokens, all heads) -> AG ===========
        ph = ExitStack()
        wv = ph.enter_context(tc.tile_pool(name="wv", bufs=2))
        psD = ph.enter_context(tc.tile_pool(name="psD", bufs=2, space="PSUM"))
        evp = ph.enter_context(tc.tile_pool(name="evp", bufs=3))

        for vc in range(8):             # 8 chunks of 512 v-columns
            wvs = wv.tile([128, KVT, CH], BF16, tag="wvs")
            nc.sync.dma_start(
                wvs[:], gwbv[:, CH * vc:CH * (vc + 1)]
                .rearrange("(l p) c -> p l c", p=128))
            for tt in range(2):         # 2 token tiles of 128
                ps = psD.tile([128, CH], F32, tag="vps")
                for lt in range(KVT):
                    nc.tensor.matmul(
                        ps[:], latkv_n[:, lt, 128 * tt:128 * (tt + 1)],
                        wvs[:, lt, :], start=(lt == 0), stop=(lt == KVT - 1))
                ev = evp.tile([128, CH], BF16, tag="vev")
                nc.scalar.copy(ev[:], ps[:])
                nc.sync.dma_start(
                    vshard[128 * tt:128 * (tt + 1), CH * vc:CH * (vc + 1)],
                    ev[:])

        ag(vshard[:], vfull[:])

        # =========== phase E: K^T (this core's 4 heads, all tokens) -> AG ====
        wkn = ph.enter_context(tc.tile_pool(name="wkn", bufs=1))
        wkns = wkn.tile([128, KVT, 4 * DN], BF16)
        nc.sync.dma_start(wkns[:], wbn_d.rearrange("p (l c) -> p l c", l=KVT))

        agp = ph.enter_context(tc.tile_pool(name="agp", bufs=2))
        for r in range(NCORES):
            slab = agp.tile([128, KVT, OWN], BF16, tag="agslab")
            nc.sync.dma_start(
                slab[:], agkv[(KVL + DR) * r:(KVL + DR) * r + KVL, :]
                .rearrange("(l p) t -> p l t", p=128))
            for hl in range(4):
                ps = psD.tile([128, OWN], F32, tag="ktps")
                for lt in range(KVT):
                    nc.tensor.matmul(ps[:], wkns[:, lt, DN * hl:DN * (hl + 1)],
                                     slab[:, lt, :],
                                     start=(lt == 0), stop=(lt == KVT - 1))
                ev = evp.tile([128, OWN], BF16, tag="ktev")
                nc.scalar.copy(ev[:], ps[:])
                # token-ordered columns: chunk r covers blocks r and 15-r
                nc.sync.dma_start(
                    ktshard[DN * hl:DN * (hl + 1), 128 * r:128 * (r + 1)],
                    ev[:, 0:128])
                nc.sync.dma_start(
                    ktshard[DN * hl:DN * (hl + 1),
                            128 * (15 - r):128 * (16 - r)],
                    ev[:, 128:256])

        ag(ktshard[:], ktfull[:])
        nc.sync.dma_start(swo[:], wo_d[:])
        ag(swo[:], gwo[:])

        # k_pe^T assembly (token-ordered, shared across heads)
        kpeT = const.tile([128, T], BF16)
        for b in range(16):
            rb = min(b, 15 - b)
            colsl = slice(0, 128) if b < 8 else slice(128, 256)
            src_ap = agkv[(KVL + DR) * rb + KVL:(KVL + DR) * rb + KVL + DR,
                          colsl]
            nc.sync.dma_start(kpeT[0:DR, 128 * b:128 * (b + 1)], src_ap)
            nc.sync.dma_start(kpeT[64:64 + DR, 128 * b:128 * (b + 1)], src_ap)
        ph.close()

        # =========== phase F: Q up-projection + rope (all heads) ===========
        qp_pool = st.enter_context(tc.tile_pool(name="qp", bufs=1))
        qTn = qp_pool.tile([128, H, OWN], BF16)
        qTp = qp_pool.tile([128, H // 2, OWN], BF16)
        attnT = qp_pool.tile([128, H, OWN], F32R)

        ph = ExitStack()
        wqb = ph.enter_context(tc.tile_pool(name="wqb", bufs=2))
        psF = ph.enter_context(tc.tile_pool(name="psF", bufs=3, space="PSUM"))
        rp = ph.enter_context(tc.tile_pool(name="rp", bufs=3))

        for h in range(H):
            ws = wqb.tile([128, QLT, DQK], BF16, tag="wqbs")
            nc.sync.dma_start(
                ws[:], gwqb[128 * h:128 * (h + 1), :]
                .rearrange("p (l c) -> p l c", l=QLT))
            pb = 0 if h < 16 else 64
            hs_ = h % 16
            psn = psF.tile([128, OWN], F32, tag="qnps")
            psp = psF.tile([128, OWN], F32, tag="qpps")
            for lt in range(QLT):
                nc.tensor.matmul(psn[:], ws[:, lt, 0:DN], latq_n[:, lt, :],
                                 start=(lt == 0), stop=(lt == QLT - 1))
            for lt in range(QLT):
                nc.tensor.matmul(psp[0:DR, :], ws[:, lt, DN:DQK],
                                 latq_n[:, lt, :],
                                 start=(lt == 0), stop=(lt == QLT - 1))
            nc.scalar.copy(qTn[:, h, :], psn[:])
            praw = rp.tile([128, OWN], F32, tag="praw")
            nc.scalar.copy(praw[0:DR, :], psp[0:DR, :])
            psw = rp.tile([128, OWN], F32, tag="psw")
            nc.sync.dma_start(psw[0:32, :], praw[32:DR, :])
            nc.sync.dma_start(psw[32:DR, :], praw[0:32, :])
            pc = rp.tile([128, OWN], F32, tag="pc")
            nc.vector.tensor_tensor(out=pc[0:DR, :], in0=praw[0:DR, :],
                                    in1=cosT[0:DR, :], op=AL.mult)
            nc.vector.tensor_tensor(out=psw[0:DR, :], in0=psw[0:DR, :],
                                    in1=sinTs[0:DR, :], op=AL.mult)
            if pb == 0:
                nc.vector.tensor_tensor(out=qTp[0:DR, hs_, :], in0=pc[0:DR, :],
                                        in1=psw[0:DR, :], op=AL.add)
            else:
                rshift = rp.tile([128, OWN], BF16, tag="rshift")
                nc.vector.tensor_tensor(out=rshift[0:DR, :], in0=pc[0:DR, :],
                                        in1=psw[0:DR, :], op=AL.add)
                nc.sync.dma_start(qTp[pb:pb + DR, hs_, :], rshift[0:DR, :])
        ph.close()

        # =========== phase G: attention ===========
        ph = ExitStack()
        ap = ph.enter_context(tc.tile_pool(name="ap", bufs=2))
        sp_pool = ph.enter_context(tc.tile_pool(name="spp", bufs=2))
        stt = ph.enter_context(tc.tile_pool(name="stt", bufs=2))
        psG = ph.enter_context(tc.tile_pool(name="psG", bufs=3, space="PSUM"))
        psT = ph.enter_context(tc.tile_pool(name="psT", bufs=2, space="PSUM"))
        psV = ph.enter_context(tc.tile_pool(name="psV", bufs=3, space="PSUM"))

        for h in range(H):
            kt = ap.tile([128, T], BF16, tag="kt")
            nc.sync.dma_start(kt[:], ktfull[DN * h:DN * (h + 1), :])
            vh = ap.tile([128, 16, DV], BF16, tag="vh")
            nc.sync.dma_start(
                vh[:], vfull.rearrange("(k p) c -> p k c", p=128)
                [:, :, DV * h:DV * (h + 1)])

            pb = 0 if h < 16 else 64
            hs_ = h % 16

            # scores per unit: nope+rope MMs, mask (in PSUM), exp -> escr + sum
            Pb = sp_pool.tile([128, 6, CH], BF16, tag="Pb")
            sumu = stt.tile([128, 6], F32, tag="sumu")
            escrs = []
            for u in (0, 1, 2, 5, 3, 4):
                q0 = 128 * UQS[u]
                ps = psG.tile([128, CH], F32, tag="sps", name=f"s{h}_{u}")
                nc.tensor.matmul(ps[:], qTn[:, h, q0:q0 + 128],
                                 kt[:, UKO[u]:UKO[u] + CH],
                                 start=True, stop=False)
                nc.tensor.matmul(ps[:], qTp[pb:pb + DR, hs_, q0:q0 + 128],
                                 kpeT[pb:pb + DR, UKO[u]:UKO[u] + CH],
                                 start=False, stop=True)
                if u in MASKED:
                    mi = MASKED.index(u)
                    nc.vector.tensor_tensor(out=ps[:], in0=ps[:],
                                            in1=mask4[:, mi, :], op=AL.add)
                escr = sp_pool.tile([128, CH], F32, tag="escr", bufs=6,
                                    name=f"e{h}_{u}")
                nc.scalar.activation(escr[:], ps[:], AF.Exp,
                                     bias=ebias[:], scale=SCALING,
                                     accum_out=sumu[:, u:u + 1])
                escrs.append(escr)

            # group sums -> diag(1/sum) matrices
            sB0 = stt.tile([128, 1], F32, tag="sB0")
            nc.vector.tensor_tensor(out=sB0[:], in0=sumu[:, 0:1],
                                    in1=sumu[:, 1:2], op=AL.add)
            sB1 = stt.tile([128, 1], F32, tag="sB1")
            nc.vector.tensor_tensor(out=sB1[:], in0=sumu[:, 2:3],
                                    in1=sumu[:, 5:6], op=AL.add)
            sB = stt.tile([128, 1], F32, tag="sB")
            nc.vector.tensor_tensor(out=sB[:], in0=sB0[:], in1=sB1[:],
                                    op=AL.add)
            sA = stt.tile([128, 1], F32, tag="sA")
            nc.vector.tensor_tensor(out=sA[:], in0=sumu[:, 3:4],
                                    in1=sumu[:, 4:5], op=AL.add)
            rA = stt.tile([128, 1], F32, tag="rA")
            nc.vector.reciprocal(rA[:], sA[:])
            rB = stt.tile([128, 1], F32, tag="rB")
            nc.vector.reciprocal(rB[:], sB[:])

            # normalize during the f32->bf16 cast (q is on partitions here)
            for i, u in enumerate((0, 1, 2, 5, 3, 4)):
                r_ = rA if u in AUNITS else rB
                if i % 2 == 0:
                    nc.vector.tensor_scalar_mul(Pb[:, u, :], escrs[i][:],
                                                r_[:])
                else:
                    nc.scalar.mul(Pb[:, u, :], escrs[i][:], r_[:])

            # P^T + PV accumulated in PSUM
            psVB = psV.tile([128, DV], F32, tag="pv", name=f"pvB{h}")
            psVA = psV.tile([128, DV], F32, tag="pv", name=f"pvA{h}")
            nmm = 0
            for u in range(6):
                isA = u in AUNITS
                dst = psVA if isA else psVB
                first = (u == 3 and True) if isA else (u == 0)
                for kb in range(4):
                    tp = psT.tile([128, 128], BF16, tag="tp")
                    nc.tensor.transpose(tp[:], Pb[:, u, 128 * kb:128 * (kb + 1)],
                                        ident[:])
                    ptT = stt.tile([128, 128], BF16, tag="ptT", bufs=4)
                    if nmm % 2 == 0:
                        nc.vector.tensor_copy(ptT[:], tp[:])
                    else:
                        nc.scalar.copy(ptT[:], tp[:])
                    nmm += 1
                    nc.tensor.matmul(
                        dst[:], vh[:, U_V[u][kb], :], ptT[:],
                        start=(first and kb == 0),
                        stop=((u == 4 and kb == 3) if isA
                              else (u == 5 and kb == 3)))
            nc.scalar.copy(attnT[:, h, 0:128], psVA[:])
            nc.vector.tensor_copy(attnT[:, h, 128:256], psVB[:])
        ph.close()

        # =========== phase H: out projection ===========
        ph = ExitStack()
        wop = ph.enter_context(tc.tile_pool(name="wop", bufs=2))
        psH = ph.enter_context(tc.tile_pool(name="psH", bufs=4, space="PSUM"))
        oev = ph.enter_context(tc.tile_pool(name="oev", bufs=3))
        abf_p = ph.enter_context(tc.tile_pool(name="abf", bufs=1))
        attnB = abf_p.tile([128, H, OWN], BF16)
        for ct in range(H):
            nc.vector.tensor_copy(attnB[:, ct, :], attnT[:, ct, :])
        for oc in range(HID // 128):
            ws = wop.tile([128, H, 128], BF16, tag="wos")
            nc.sync.dma_start(
                ws[:], gwo[128 * oc:128 * (oc + 1), :]
                .rearrange("p (t c) -> p t c", t=H))
            ps = psH.tile([128, OWN], F32, tag="ops")
            for ct in range(H):
                nc.tensor.matmul(ps[:], ws[:, ct, :], attnB[:, ct, :],
                                 start=(ct == 0), stop=(ct == H - 1))
            ev = oev.tile([128, OWN], F32, tag="oev")
            nc.scalar.copy(ev[:], ps[:])
            nc.sync.dma_start(outT_d[128 * oc:128 * (oc + 1), :], ev[:])
        ph.close()
        st.close()

    nc.finalize()
    legalize_sync_waits(nc)
    return nc


_DEINT = np.array([2 * r if r < 32 else 2 * r - 63 for r in range(DR)])


def _host_prep(inputs):
    f32 = np.float32
    bf16 = ml_dtypes.bfloat16
    hs = np.asarray(inputs["hidden_states"], f32)
    cos = np.asarray(inputs["cos"], f32).reshape(T, DR)
    sin = np.asarray(inputs["sin"], f32).reshape(T, DR)
    wq_a = np.asarray(inputs["wq_a"], f32)
    q_ln = np.asarray(inputs["q_a_ln_w"], f32)
    wq_b = np.asarray(inputs["wq_b"], f32)
    wkv_a = np.asarray(inputs["wkv_a"], f32)
    kv_ln = np.asarray(inputs["kv_a_ln_w"], f32)
    wkv_b = np.asarray(inputs["wkv_b"], f32)
    wo = np.asarray(inputs["wo"], f32)

    # fold ln weights into up-projections
    wq_b = wq_b * q_ln[:, None]
    wkv_b = wkv_b * kv_ln[:, None]

    # deinterleave fold: q_pe columns of wq_b, k_pe columns of wkv_a
    wqbp = wq_b.copy()
    for h in range(H):
        pe = wq_b[:, h * DQK + DN:h * DQK + DQK]
        wqbp[:, h * DQK + DN:h * DQK + DQK] = pe[:, _DEINT]
    wkvap = np.zeros((HID, KVT5 * 128), f32)
    wkvap[:, :KVL] = wkv_a[:, :KVL]
    wkvap[:, KVL:KVL + DR] = wkv_a[:, KVL:][:, _DEINT]

    # split wkv_b into nope / v column groups (head-major)
    wkvbn = np.concatenate(
        [wkv_b[:, h * 256:h * 256 + DN] for h in range(H)], axis=1)
    wkvbv = np.concatenate(
        [wkv_b[:, h * 256 + DN:h * 256 + 256] for h in range(H)], axis=1)

    # swizzles: slab-major, partition-contiguous layouts
    WQA = wq_a.reshape(HT, 128, QLT, 128).transpose(2, 1, 0, 3) \
        .reshape(QL, HID).astype(bf16)
    WKVA = wkvap.reshape(HT, 128, KVT5, 128).transpose(2, 1, 0, 3) \
        .reshape(KVT5 * 128, HID).astype(bf16)
    WQB = wqbp.reshape(QLT, 128, H, DQK).transpose(2, 1, 0, 3) \
        .reshape(H * 128, QLT * DQK).astype(bf16)
    WBV = wkvbv.astype(bf16)                      # [512, 4096] natural
    WO = wo.reshape(H, 128, HT, 128).transpose(2, 1, 0, 3) \
        .reshape(HID, H * DV).astype(bf16)

    cosT = np.ascontiguousarray(cos.T)
    sinT = np.ascontiguousarray(sin.T)
    sinTs = sinT.copy()
    sinTs[0:32] = -sinT[0:32]

    ident = np.eye(128, dtype=bf16)
    ones128 = np.ones((128, 1), f32)
    onesrow = np.ones((1, 128), f32)

    def shard(a):
        n = a.shape[0] // NCORES
        return [np.ascontiguousarray(a[c * n:(c + 1) * n]) for c in
                range(NCORES)]

    WQA_s, WKVA_s, WQB_s, WBV_s, WO_s = (shard(WQA), shard(WKVA), shard(WQB),
                                         shard(WBV), shard(WO))

    qr = np.arange(128)[:, None]
    kr = np.arange(CH)[None, :]

    in_maps = []
    for c in range(NCORES):
        bA, bB = c, 15 - c
        own = np.r_[np.arange(128 * bA, 128 * bA + 128),
                    np.arange(128 * bB, 128 * bB + 128)]
        # masks for units u2,u3,u4,u5
        mask4 = np.zeros((128, 4, CH), f32)
        specs = [(bB, 2 * CH, True),          # u2
                 (bA, 0, True),               # u3
                 (bA, CH, c >= 4),            # u4
                 (bB, 3 * CH, c < 4)]         # u5
        for mi, (qb, koff, active) in enumerate(specs):
            if not active:
                mask4[:, mi, :] = NEG
            else:
                qtok = 128 * qb + qr
                ktok = koff + kr
                mask4[:, mi, :] = np.where(ktok <= qtok, 0.0, NEG)

        hid_own = np.ascontiguousarray(hs[own].T)      # [5120, 256]
        hidp = hid_own.reshape(HT, 128, OWN).transpose(1, 0, 2) \
            .reshape(128, HT * OWN).astype(bf16)

        wbn_c = wkvbn[:, 4 * DN * c:4 * DN * (c + 1)]  # [512, 512]
        wbn = wbn_c.reshape(KVT, 128, 4 * DN).transpose(1, 0, 2) \
            .reshape(128, KVT * 4 * DN).astype(bf16)

        in_maps.append({
            "hidp": hidp,
            "cosT": np.ascontiguousarray(cosT[:, own]),
            "sinTs": np.ascontiguousarray(sinTs[:, own]),
            "wqa_sh": WQA_s[c],
            "wkva_sh": WKVA_s[c],
            "wqb_sh": WQB_s[c],
            "wbv_sh": WBV_s[c],
            "wo_sh": WO_s[c],
            "wbn": wbn,
            "mask4": mask4,
            "ident": ident, "ones128": ones128, "onesrow": onesrow,
        })
    return in_maps


_NC_CACHE = None


def _get_nc():
    global _NC_CACHE
    if _NC_CACHE is None:
        _NC_CACHE = build_bass()
    return _NC_CACHE


def run(inputs, trace=False):
    nc = _get_nc()
    in_maps = _host_prep(inputs)
    res = run_bass_kernel_spmd(nc, in_maps, list(range(NCORES)), trace=trace)
    out = np.empty((T, HID), np.float32)
    for c in range(NCORES):
        oT = res.results[c]["outT"]
        out[128 * c:128 * (c + 1)] = oT[:, 0:128].T
        out[128 * (15 - c):128 * (16 - c)] = oT[:, 128:256].T
    return out, res


def kernel(**inputs):
    out, _ = run(inputs, trace=False)
    return out


# revision 14
# speedup vs baseline: 1.9619x; 1.1243x over previous
"""DeepseekV2 MLA prefill attention on 8 NeuronCores (Trainium2, Bass/Tile).

Sharding: token-parallel attention with zigzag blocks (core c owns token
blocks {c, 15-c}); all large weights are uploaded row-sharded (1/8 per core)
and broadcast on-device via AllGather into internal DRAM, so every weight
byte crosses the host link exactly once. Weight tensors are pre-swizzled on
host so each SBUF slab load is partition-contiguous (KB-sized descriptors).

Attention softmax uses a constant exp bias (scores are bounded; verified
max |scale*s| ~ 5.5 << 16) so exp reads score PSUM directly -- no row-max
pass, no second exp pass. Normalization is folded into the P^T transpose by
multiplying against diag(1/rowsum) instead of the identity. P@V accumulates
across units in PSUM.

SPMD: one program for all cores; per-core variation carried by mask data.
"""
import sys
import json

sys.path.insert(0, "/opt/trn_rl_repo")

import numpy as np
import ml_dtypes

import concourse.bass as bass
import concourse.mybir as mybir
import concourse.tile as tile
from concourse.bass_utils import run_bass_kernel_spmd

F32 = mybir.dt.float32
F32R = mybir.dt.float32r
BF16 = mybir.dt.bfloat16

T = 2048
H = 32
HID = 5120
QL = 1536
KVL = 512
DN = 128
DR = 64
DQK = DN + DR
DV = 128
EPS = 1e-6
SCALING = DQK ** -0.5
NCORES = 8
OWN = 256
CH = 512
NEG = -1e30
EB = 16.0            # constant exp bias: exp(SCALING*s - EB)

HT = HID // 128      # 40
QLT = QL // 128      # 12
KVT = KVL // 128     # 4
KVT5 = KVT + 1       # 4 latent slabs + 1 (padded) rope slab

# attention units: u0-u2 = B-tile keys [0:512/512:1024/1024:1536],
# u3 = A-tile keys [0:512], u4 = A-tile keys [512:1024] (active c>=4),
# u5 = B-tile keys [1536:2048] (active c<4). Inactive units fully masked.
UQS = [1, 1, 1, 0, 0, 1]             # 1 = B tile (q cols 128:256)
UKO = [0, CH, 2 * CH, 0, CH, 3 * CH]
MASKED = [2, 3, 4, 5]                # units with an additive mask
AUNITS = (3, 4)
BUNITS = (0, 1, 2, 5)
# vh slot lists per unit (vfull rows are rank-major: slot k=2r+s ~ block
# b = r (s=0) or 15-r (s=1); token block b -> slot 2b if b<8 else 2(15-b)+1)
U_V = [[0, 2, 4, 6], [8, 10, 12, 14], [15, 13, 11, 9],
       [0, 2, 4, 6], [8, 10, 12, 14], [7, 5, 3, 1]]


def legalize_sync_waits(nc):
    """This container's walrus accepts at most one sync-wait per instruction;
    split extras onto standalone EventSemaphore waits just before (same
    engine; engine streams preserve intra-block order)."""
    m = json.loads(nc.to_json_bytes())
    ctr = [0]

    def fresh():
        ctr[0] += 1
        return f"I-lw-{ctr[0]}"

    for f in m["functions"]:
        for bb in f["blocks"]:
            out = []
            for ins in bb["instructions"]:
                si = ins.get("sync_info")
                waits = (si or {}).get("on_wait") or []
                if len(waits) > 1:
                    for w in waits[:-1]:
                        out.append({
                            "debug": ins.get("debug", 0),
                            "engine": ins["engine"],
                            "ins": [], "outs": [],
                            "name": fresh(),
                            "opcode": "EventSemaphore",
                            "sync_info": {"on_update": [], "on_wait": [w]},
                        })
                    si["on_wait"] = waits[-1:]
                out.append(ins)
            bb["instructions"] = out
    nc.m = mybir.module_from_json_bytes(json.dumps(m).encode())
    return nc


def build_bass():
    nc = bass.Bass()
    AL = mybir.AluOpType
    AF = mybir.ActivationFunctionType

    dp = nc.declare_dram_parameter
    hidp_d = dp("hidp", [128, HT * OWN], BF16, isOutput=False)
    cosT_d = dp("cosT", [DR, OWN], F32, isOutput=False)
    sinTs_d = dp("sinTs", [DR, OWN], F32, isOutput=False)
    wqa_d = dp("wqa_sh", [QL // 8, HID], BF16, isOutput=False)
    wkva_d = dp("wkva_sh", [KVT5 * 128 // 8, HID], BF16, isOutput=False)
    wqb_d = dp("wqb_sh", [H * 128 // 8, QLT * DQK], BF16, isOutput=False)
    wbv_d = dp("wbv_sh", [KVL // 8, H * DV], BF16, isOutput=False)
    wo_d = dp("wo_sh", [HID // 8, H * DV], BF16, isOutput=False)
    wbn_d = dp("wbn", [128, KVT * 4 * DN], BF16, isOutput=False)
    mask4_d = dp("mask4", [128, 4, CH], F32, isOutput=False)
    ident_d = dp("ident", [128, 128], BF16, isOutput=False)
    ones128_d = dp("ones128", [128, 1], F32R, isOutput=False)
    onesrow_d = dp("onesrow", [1, 128], F32, isOutput=False)
    outT_d = dp("outT", [HID, OWN], F32, isOutput=True)

    RG = [list(range(NCORES))]

    with tile.TileContext(nc) as tc:
        from contextlib import ExitStack
        st = ExitStack()
        const = st.enter_context(tc.tile_pool(name="const", bufs=1))
        dram = st.enter_context(tc.tile_pool(name="dram", bufs=1, space="DRAM"))

        # ---- AG destinations (internal DRAM, Shared) ----
        gwqa = dram.tile([QL, HID], BF16, addr_space="Shared")
        gwkva = dram.tile([KVT5 * 128, HID], BF16, addr_space="Shared")
        gwqb = dram.tile([H * 128, QLT * DQK], BF16, addr_space="Shared")
        gwbv = dram.tile([KVL, H * DV], BF16, addr_space="Shared")
        gwo = dram.tile([HID, H * DV], BF16, addr_space="Shared")
        agin = dram.tile([KVL + DR, OWN], BF16)
        agkv = dram.tile([NCORES * (KVL + DR), OWN], BF16, addr_space="Shared")
        vshard = dram.tile([OWN, H * DV], BF16)
        vfull = dram.tile([T, H * DV], BF16, addr_space="Shared")
        ktshard = dram.tile([4 * DN, T], BF16)
        ktfull = dram.tile([H * DN, T], BF16, addr_space="Shared")

        def ag(inp, outp):
            nc.gpsimd.collective_compute(
                "AllGather", AL.bypass, replica_groups=RG,
                ins=[inp.opt()], outs=[outp.opt()])

        # weight broadcasts: stage each shard param into internal DRAM
        # (collectives cannot read IO tensors), then AllGather. Ordered by
        # first use so the CC queue never blocks a consumer longer than
        # needed.
        swqa = dram.tile([QL // 8, HID], BF16)
        swkva = dram.tile([KVT5 * 128 // 8, HID], BF16)
        swqb = dram.tile([H * 128 // 8, QLT * DQK], BF16)
        swbv = dram.tile([KVL // 8, H * DV], BF16)
        swo = dram.tile([HID // 8, H * DV], BF16)
        nc.sync.dma_start(swkva[:], wkva_d[:])
        nc.sync.dma_start(swqa[:], wqa_d[:])
        nc.sync.dma_start(swbv[:], wbv_d[:])
        nc.sync.dma_start(swqb[:], wqb_d[:])
        nc.sync.dma_start(swo[1:HID // 8, :], wo_d[1:HID // 8, :])
        ag(swkva[:], gwkva[:])
        ag(swqa[:], gwqa[:])
        ag(swbv[:], gwbv[:])
        ag(swqb[:], gwqb[:])

        # ---- constants ----
        ident = const.tile([128, 128], BF16)
        nc.sync.dma_start(ident[:], ident_d[:])
        ones128 = const.tile([128, 1], F32R)
        nc.sync.dma_start(ones128[:], ones128_d[:])
        onesrow = const.tile([1, 128], F32)
        nc.sync.dma_start(onesrow[:], onesrow_d[:])
        mask4 = const.tile([128, 4, CH], F32)
        nc.sync.dma_start(mask4[:], mask4_d[:])
        cosT = const.tile([128, OWN], F32)
        nc.sync.dma_start(cosT[0:DR, :], cosT_d[:])
        nc.sync.dma_start(cosT[64:64 + DR, :], cosT_d[:])
        sinTs = const.tile([128, OWN], F32)
        nc.sync.dma_start(sinTs[0:DR, :], sinTs_d[:])
        nc.sync.dma_start(sinTs[64:64 + DR, :], sinTs_d[:])
        epsc = const.tile([1, 1], F32)
        nc.vector.memset(epsc[:], EPS)
        ebias = const.tile([128, 1], F32)
        nc.vector.memset(ebias[:], -EB)

        # =========== phase B: down projections (transposed) ===========
        latp = st.enter_context(tc.tile_pool(name="latp", bufs=1))
        ph = ExitStack()
        hidp = ph.enter_context(tc.tile_pool(name="hidp", bufs=1))
        wsl = ph.enter_context(tc.tile_pool(name="wsl", bufs=2))
        rawp = ph.enter_context(tc.tile_pool(name="rawp", bufs=1))
        psB = ph.enter_context(tc.tile_pool(name="psB", bufs=4, space="PSUM"))
        psS = ph.enter_context(tc.tile_pool(name="psS", bufs=2, space="PSUM"))

        hidT = hidp.tile([128, HT, OWN], BF16)
        nc.sync.dma_start(hidT[:], hidp_d.rearrange("p (a t) -> p a t", a=HT))

        latq = rawp.tile([128, QLT, OWN], F32)
        latkv = rawp.tile([128, KVT5, OWN], F32)

        for lt in range(KVT5):
            wslab = wsl.tile([128, HT, 128], BF16, tag="wslab")
            nc.sync.dma_start(
                wslab[:], gwkva[128 * lt:128 * (lt + 1), :]
                .rearrange("p (a c) -> p a c", a=HT))
            ps = psB.tile([128, OWN], F32, tag="dps")
            for ht in range(HT):
                nc.tensor.matmul(ps[:], wslab[:, ht, :], hidT[:, ht, :],
                                 start=(ht == 0), stop=(ht == HT - 1))
            nc.scalar.copy(latkv[:, lt, :], ps[:])

        for lt in range(QLT):
            wslab = wsl.tile([128, HT, 128], BF16, tag="wslab")
            nc.sync.dma_start(
                wslab[:], gwqa[128 * lt:128 * (lt + 1), :]
                .rearrange("p (a c) -> p a c", a=HT))
            ps = psB.tile([128, OWN], F32, tag="dps")
            for ht in range(HT):
                nc.tensor.matmul(ps[:], wslab[:, ht, :], hidT[:, ht, :],
                                 start=(ht == 0), stop=(ht == HT - 1))
            nc.scalar.copy(latq[:, lt, :], ps[:])

        # ---- rmsnorm factors via squares + ones-matmul ----
        latq_n = latp.tile([128, QLT, OWN], BF16)
        latkv_n = latp.tile([128, KVT, OWN], BF16)

        def rmsnorm(lat, lat_n, nt, L):
            ssq = psS.tile([1, OWN], F32, tag="ssq")
            for lt in range(nt):
                sq = rawp.tile([128, OWN], F32R, tag="sqscratch", bufs=2)
                nc.vector.tensor_tensor(out=sq[:], in0=lat[:, lt, :],
                                        in1=lat[:, lt, :], op=AL.mult)
                nc.tensor.matmul(ssq[:], ones128[:], sq[:],
                                 start=(lt == 0), stop=(lt == nt - 1))
            f = rawp.tile([1, OWN], F32, tag="fscratch", bufs=2)
            nc.scalar.activation(f[:], ssq[:], AF.Sqrt, bias=epsc[:],
                                 scale=1.0 / L)
            fr = rawp.tile([1, OWN], F32, tag="frscratch", bufs=2)
            nc.vector.reciprocal(fr[:], f[:])
            fb = psS.tile([128, OWN], F32, tag="fbcast")
            nc.tensor.matmul(fb[:], onesrow[:], fr[:], start=True, stop=True)
            for lt in range(nt):
                nc.vector.tensor_tensor(out=lat_n[:, lt, :], in0=lat[:, lt, :],
                                        in1=fb[:], op=AL.mult)

        rmsnorm(latkv, latkv_n, KVT, KVL)
        rmsnorm(latq, latq_n, QLT, QL)

        # ---- rope k_pe (deinterleave folded into wkva on host) ----
        kpsw = rawp.tile([128, OWN], F32)
        nc.sync.dma_start(kpsw[0:32, :], latkv[32:64, KVT, :])
        nc.sync.dma_start(kpsw[32:64, :], latkv[0:32, KVT, :])
        kpc = rawp.tile([128, OWN], F32)
        nc.vector.tensor_tensor(out=kpc[0:DR, :], in0=latkv[0:DR, KVT, :],
                                in1=cosT[0:DR, :], op=AL.mult)
        nc.vector.tensor_tensor(out=kpsw[0:DR, :], in0=kpsw[0:DR, :],
                                in1=sinTs[0:DR, :], op=AL.mult)
        kpeR = rawp.tile([128, OWN], BF16)
        nc.vector.tensor_tensor(out=kpeR[0:DR, :], in0=kpc[0:DR, :],
                                in1=kpsw[0:DR, :], op=AL.add)

        # assemble AG input: rows 0:512 normalized latent, 512:576 roped kpe
        for lt in range(KVT):
            nc.sync.dma_start(agin[128 * lt:128 * (lt + 1), :],
                              latkv_n[:, lt, :])
        nc.sync.dma_start(agin[KVL:KVL + DR, :], kpeR[0:DR, :])
        ag(agin[:], agkv[:])
        ph.close()

        # =========== phase D: V (own tokens, all heads) -> AG ===========
        ph = ExitStack()
        wv = ph.enter_context(tc.tile_pool(name="wv", bufs=2))
        psD = ph.enter_context(tc.tile_pool(name="psD", bufs=2, space="PSUM"))
        evp = ph.enter_context(tc.tile_pool(name="evp", bufs=3))

        for vc in range(8):             # 8 chunks of 512 v-columns
            wvs = wv.tile([128, KVT, CH], BF16, tag="wvs")
            nc.sync.dma_start(
                wvs[:], gwbv[:, CH * vc:CH * (vc + 1)]
                .rearrange("(l p) c -> p l c", p=128))
            for tt in range(2):         # 2 token tiles of 128
                ps = psD.tile([128, CH], F32, tag="vps")
                for lt in range(KVT):
                    nc.tensor.matmul(
                        ps[:], latkv_n[:, lt, 128 * tt:128 * (tt + 1)],
                        wvs[:, lt, :], start=(lt == 0), stop=(lt == KVT - 1))
                ev = evp.tile([128, CH], BF16, tag="vev")
                nc.scalar.copy(ev[:], ps[:])
                nc.sync.dma_start(
                    vshard[128 * tt:128 * (tt + 1), CH * vc:CH * (vc + 1)],
                    ev[:])

        ag(vshard[:], vfull[:])

        # =========== phase E: K^T (this core's 4 heads, all tokens) -> AG ====
        wkn = ph.enter_context(tc.tile_pool(name="wkn", bufs=1))
        wkns = wkn.tile([128, KVT, 4 * DN], BF16)
        nc.sync.dma_start(wkns[:], wbn_d.rearrange("p (l c) -> p l c", l=KVT))

        agp = ph.enter_context(tc.tile_pool(name="agp", bufs=2))
        for r in range(NCORES):
            slab = agp.tile([128, KVT, OWN], BF16, tag="agslab")
            nc.sync.dma_start(
                slab[:], agkv[(KVL + DR) * r:(KVL + DR) * r + KVL, :]
                .rearrange("(l p) t -> p l t", p=128))
            for hl in range(4):
                ps = psD.tile([128, OWN], F32, tag="ktps")
                for lt in range(KVT):
                    nc.tensor.matmul(ps[:], wkns[:, lt, DN * hl:DN * (hl + 1)],
                                     slab[:, lt, :],
                                     start=(lt == 0), stop=(lt == KVT - 1))
                ev = evp.tile([128, OWN], BF16, tag="ktev")
                nc.scalar.copy(ev[:], ps[:])
                # token-ordered columns: chunk r covers blocks r and 15-r
                nc.sync.dma_start(
                    ktshard[DN * hl:DN * (hl + 1), 128 * r:128 * (r + 1)],
                    ev[:, 0:128])
                nc.sync.dma_start(
                    ktshard[DN * hl:DN * (hl + 1),
                            128 * (15 - r):128 * (16 - r)],
                    ev[:, 128:256])

        ag(ktshard[:], ktfull[:])
        wob = agp.tile([1, H * DV], BF16, tag="agslab")
        nc.sync.dma_start(wob[:], wo_d[0:1, :])
        nc.sync.dma_start(swo[0:1, :], wob[:])
        ag(swo[:], gwo[:])

        # k_pe^T assembly (token-ordered, shared across heads)
        kpeT = const.tile([128, T], BF16)
        for b in range(16):
            rb = min(b, 15 - b)
            colsl = slice(0, 128) if b < 8 else slice(128, 256)
            src_ap = agkv[(KVL + DR) * rb + KVL:(KVL + DR) * rb + KVL + DR,
                          colsl]
            nc.sync.dma_start(kpeT[0:DR, 128 * b:128 * (b + 1)], src_ap)
            nc.sync.dma_start(kpeT[64:64 + DR, 128 * b:128 * (b + 1)], src_ap)
        ph.close()

        # =========== phase F: Q up-projection + rope (all heads) ===========
        qp_pool = st.enter_context(tc.tile_pool(name="qp", bufs=1))
        qTn = qp_pool.tile([128, H, OWN], BF16)
        qTp = qp_pool.tile([128, H // 2, OWN], BF16)
        attnT = qp_pool.tile([128, H, OWN], F32R)

        ph = ExitStack()
        wqb = ph.enter_context(tc.tile_pool(name="wqb", bufs=2))
        psF = ph.enter_context(tc.tile_pool(name="psF", bufs=3, space="PSUM"))
        rp = ph.enter_context(tc.tile_pool(name="rp", bufs=3))

        for h in range(H):
            ws = wqb.tile([128, QLT, DQK], BF16, tag="wqbs")
            nc.sync.dma_start(
                ws[:], gwqb[128 * h:128 * (h + 1), :]
                .rearrange("p (l c) -> p l c", l=QLT))
            pb = 0 if h < 16 else 64
            hs_ = h % 16
            psn = psF.tile([128, OWN], F32, tag="qnps")
            psp = psF.tile([128, OWN], F32, tag="qpps")
            for lt in range(QLT):
                nc.tensor.matmul(psn[:], ws[:, lt, 0:DN], latq_n[:, lt, :],
                                 start=(lt == 0), stop=(lt == QLT - 1))
            for lt in range(QLT):
                nc.tensor.matmul(psp[0:DR, :], ws[:, lt, DN:DQK],
                                 latq_n[:, lt, :],
                                 start=(lt == 0), stop=(lt == QLT - 1))
            nc.scalar.copy(qTn[:, h, :], psn[:])
            praw = rp.tile([128, OWN], F32, tag="praw")
            nc.scalar.copy(praw[0:DR, :], psp[0:DR, :])
            psw = rp.tile([128, OWN], F32, tag="psw")
            nc.sync.dma_start(psw[0:32, :], praw[32:DR, :])
            nc.sync.dma_start(psw[32:DR, :], praw[0:32, :])
            pc = rp.tile([128, OWN], F32, tag="pc")
            nc.vector.tensor_tensor(out=pc[0:DR, :], in0=praw[0:DR, :],
                                    in1=cosT[0:DR, :], op=AL.mult)
            nc.vector.tensor_tensor(out=psw[0:DR, :], in0=psw[0:DR, :],
                                    in1=sinTs[0:DR, :], op=AL.mult)
            if pb == 0:
                nc.vector.tensor_tensor(out=qTp[0:DR, hs_, :], in0=pc[0:DR, :],
                                        in1=psw[0:DR, :], op=AL.add)
            else:
                rshift = rp.tile([128, OWN], BF16, tag="rshift")
                nc.vector.tensor_tensor(out=rshift[0:DR, :], in0=pc[0:DR, :],
                                        in1=psw[0:DR, :], op=AL.add)
                nc.sync.dma_start(qTp[pb:pb + DR, hs_, :], rshift[0:DR, :])
        ph.close()

        # =========== phase G: attention ===========
        ph = ExitStack()
        ap = ph.enter_context(tc.tile_pool(name="ap", bufs=2))
        sp_pool = ph.enter_context(tc.tile_pool(name="spp", bufs=2))
        stt = ph.enter_context(tc.tile_pool(name="stt", bufs=2))
        psG = ph.enter_context(tc.tile_pool(name="psG", bufs=2, space="PSUM"))
        psT = ph.enter_context(tc.tile_pool(name="psT", bufs=4, space="PSUM"))
        psV = ph.enter_context(tc.tile_pool(name="psV", bufs=2, space="PSUM"))

        for h in range(H):
            kt = ap.tile([128, T], BF16, tag="kt")
            nc.sync.dma_start(kt[:], ktfull[DN * h:DN * (h + 1), :])
            vh = ap.tile([128, 16, DV], BF16, tag="vh")
            nc.sync.dma_start(
                vh[:], vfull.rearrange("(k p) c -> p k c", p=128)
                [:, :, DV * h:DV * (h + 1)])

            pb = 0 if h < 16 else 64
            hs_ = h % 16

            # scores per unit: nope+rope MMs, mask (in PSUM), exp -> escr + sum
            Pb = sp_pool.tile([128, 6, CH], BF16, tag="Pb")
            sumu = stt.tile([128, 6], F32, tag="sumu")
            escrs = []
            for u in (0, 1, 2, 5, 3, 4):
                q0 = 128 * UQS[u]
                ps = psG.tile([128, CH], F32, tag="sps", name=f"s{h}_{u}")
                nc.tensor.matmul(ps[:], qTn[:, h, q0:q0 + 128],
                                 kt[:, UKO[u]:UKO[u] + CH],
                                 start=True, stop=False)
                nc.tensor.matmul(ps[:], qTp[pb:pb + DR, hs_, q0:q0 + 128],
                                 kpeT[pb:pb + DR, UKO[u]:UKO[u] + CH],
                                 start=False, stop=True)
                if u in MASKED:
                    mi = MASKED.index(u)
                    nc.vector.tensor_tensor(out=ps[:], in0=ps[:],
                                            in1=mask4[:, mi, :], op=AL.add)
                escr = sp_pool.tile([128, CH], F32, tag="escr", bufs=8,
                                    name=f"e{h}_{u}")
                nc.scalar.activation(escr[:], ps[:], AF.Exp,
                                     bias=ebias[:], scale=SCALING,
                                     accum_out=sumu[:, u:u + 1])
                escrs.append(escr)

            # group sums -> diag(1/sum) matrices
            sB0 = stt.tile([128, 1], F32, tag="sB0")
            nc.vector.tensor_tensor(out=sB0[:], in0=sumu[:, 0:1],
                                    in1=sumu[:, 1:2], op=AL.add)
            sB1 = stt.tile([128, 1], F32, tag="sB1")
            nc.vector.tensor_tensor(out=sB1[:], in0=sumu[:, 2:3],
                                    in1=sumu[:, 5:6], op=AL.add)
            sB = stt.tile([128, 1], F32, tag="sB")
            nc.vector.tensor_tensor(out=sB[:], in0=sB0[:], in1=sB1[:],
                                    op=AL.add)
            sA = stt.tile([128, 1], F32, tag="sA")
            nc.vector.tensor_tensor(out=sA[:], in0=sumu[:, 3:4],
                                    in1=sumu[:, 4:5], op=AL.add)
            rA = stt.tile([128, 1], F32, tag="rA")
            nc.vector.reciprocal(rA[:], sA[:])
            rB = stt.tile([128, 1], F32, tag="rB")
            nc.vector.reciprocal(rB[:], sB[:])

            # normalize during the f32->bf16 cast (q is on partitions here)
            for i, u in enumerate((0, 1, 2, 5, 3, 4)):
                r_ = rA if u in AUNITS else rB
                if i % 3 == 2:
                    nc.scalar.mul(Pb[:, u, :], escrs[i][:], r_[:])
                else:
                    nc.vector.tensor_scalar_mul(Pb[:, u, :], escrs[i][:],
                                                r_[:])

            # P^T + PV accumulated in PSUM
            psVB = psV.tile([128, DV], F32, tag="pv", name=f"pvB{h}")
            psVA = psV.tile([128, DV], F32, tag="pv", name=f"pvA{h}")
            nmm = 0
            for u in range(6):
                isA = u in AUNITS
                dst = psVA if isA else psVB
                first = (u == 3 and True) if isA else (u == 0)
                for kb in range(4):
                    tp = psT.tile([128, 128], BF16, tag="tp")
                    nc.tensor.transpose(tp[:], Pb[:, u, 128 * kb:128 * (kb + 1)],
                                        ident[:])
                    ptT = stt.tile([128, 128], BF16, tag="ptT", bufs=4)
                    if nmm % 3 == 2:
                        nc.scalar.copy(ptT[:], tp[:])
                    else:
                        nc.vector.tensor_copy(ptT[:], tp[:])
                    nmm += 1
                    nc.tensor.matmul(
                        dst[:], vh[:, U_V[u][kb], :], ptT[:],
                        start=(first and kb == 0),
                        stop=((u == 4 and kb == 3) if isA
                              else (u == 5 and kb == 3)))
            nc.scalar.copy(attnT[:, h, 0:128], psVA[:])
            nc.vector.tensor_copy(attnT[:, h, 128:256], psVB[:])
        ph.close()

        # =========== phase H: out projection ===========
        ph = ExitStack()
        wop = ph.enter_context(tc.tile_pool(name="wop", bufs=2))
        psH = ph.enter_context(tc.tile_pool(name="psH", bufs=4, space="PSUM"))
        oev = ph.enter_context(tc.tile_pool(name="oev", bufs=3))
        abf_p = ph.enter_context(tc.tile_pool(name="abf", bufs=1))
        attnB = abf_p.tile([128, H, OWN], BF16)
        for ct in range(H):
            nc.vector.tensor_copy(attnB[:, ct, :], attnT[:, ct, :])
        for oc in range(HID // 128):
            ws = wop.tile([128, H, 128], BF16, tag="wos")
            nc.sync.dma_start(
                ws[:], gwo[128 * oc:128 * (oc + 1), :]
                .rearrange("p (t c) -> p t c", t=H))
            ps = psH.tile([128, OWN], F32, tag="ops")
            for ct in range(H):
                nc.tensor.matmul(ps[:], ws[:, ct, :], attnB[:, ct, :],
                                 start=(ct == 0), stop=(ct == H - 1))
            ev = oev.tile([128, OWN], F32, tag="oev")
            nc.scalar.copy(ev[:], ps[:])
            nc.sync.dma_start(outT_d[128 * oc:128 * (oc + 1), :], ev[:])
        ph.close()
        st.close()

    nc.finalize()
    legalize_sync_waits(nc)
    return nc


_DEINT = np.array([2 * r if r < 32 else 2 * r - 63 for r in range(DR)])


def _host_prep(inputs):
    f32 = np.float32
    bf16 = ml_dtypes.bfloat16
    hs = np.asarray(inputs["hidden_states"], f32)
    cos = np.asarray(inputs["cos"], f32).reshape(T, DR)
    sin = np.asarray(inputs["sin"], f32).reshape(T, DR)
    wq_a = np.asarray(inputs["wq_a"], f32)
    q_ln = np.asarray(inputs["q_a_ln_w"], f32)
    wq_b = np.asarray(inputs["wq_b"], f32)
    wkv_a = np.asarray(inputs["wkv_a"], f32)
    kv_ln = np.asarray(inputs["kv_a_ln_w"], f32)
    wkv_b = np.asarray(inputs["wkv_b"], f32)
    wo = np.asarray(inputs["wo"], f32)

    # fold ln weights into up-projections
    wq_b = wq_b * q_ln[:, None]
    wkv_b = wkv_b * kv_ln[:, None]

    # deinterleave fold: q_pe columns of wq_b, k_pe columns of wkv_a
    wqbp = wq_b.copy()
    for h in range(H):
        pe = wq_b[:, h * DQK + DN:h * DQK + DQK]
        wqbp[:, h * DQK + DN:h * DQK + DQK] = pe[:, _DEINT]
    wkvap = np.zeros((HID, KVT5 * 128), f32)
    wkvap[:, :KVL] = wkv_a[:, :KVL]
    wkvap[:, KVL:KVL + DR] = wkv_a[:, KVL:][:, _DEINT]

    # split wkv_b into nope / v column groups (head-major)
    wkvbn = np.concatenate(
        [wkv_b[:, h * 256:h * 256 + DN] for h in range(H)], axis=1)
    wkvbv = np.concatenate(
        [wkv_b[:, h * 256 + DN:h * 256 + 256] for h in range(H)], axis=1)

    # swizzles: slab-major, partition-contiguous layouts
    WQA = wq_a.reshape(HT, 128, QLT, 128).transpose(2, 1, 0, 3) \
        .reshape(QL, HID).astype(bf16)
    WKVA = wkvap.reshape(HT, 128, KVT5, 128).transpose(2, 1, 0, 3) \
        .reshape(KVT5 * 128, HID).astype(bf16)
    WQB = wqbp.reshape(QLT, 128, H, DQK).transpose(2, 1, 0, 3) \
        .reshape(H * 128, QLT * DQK).astype(bf16)
    WBV = wkvbv.astype(bf16)                      # [512, 4096] natural
    WO = wo.reshape(H, 128, HT, 128).transpose(2, 1, 0, 3) \
        .reshape(HID, H * DV).astype(bf16)

    cosT = np.ascontiguousarray(cos.T)
    sinT = np.ascontiguousarray(sin.T)
    sinTs = sinT.copy()
    sinTs[0:32] = -sinT[0:32]

    ident = np.eye(128, dtype=bf16)
    ones128 = np.ones((128, 1), f32)
    onesrow = np.ones((1, 128), f32)

    def shard(a):
        n = a.shape[0] // NCORES
        return [np.ascontiguousarray(a[c * n:(c + 1) * n]) for c in
                range(NCORES)]

    WQA_s, WKVA_s, WQB_s, WBV_s, WO_s = (shard(WQA), shard(WKVA), shard(WQB),
                                         shard(WBV), shard(WO))

    qr = np.arange(128)[:, None]
    kr = np.arange(CH)[None, :]

    in_maps = []
    for c in range(NCORES):
        bA, bB = c, 15 - c
        own = np.r_[np.arange(128 * bA, 128 * bA + 128),
                    np.arange(128 * bB, 128 * bB + 128)]
        # masks for units u2,u3,u4,u5
        mask4 = np.zeros((128, 4, CH), f32)
        specs = [(bB, 2 * CH, True),          # u2
                 (bA, 0, True),               # u3
                 (bA, CH, c >= 4),            # u4
                 (bB, 3 * CH, c < 4)]         # u5
        for mi, (qb, koff, active) in enumerate(specs):
            if not active:
                mask4[:, mi, :] = NEG
            else:
                qtok = 128 * qb + qr
                ktok = koff + kr
                mask4[:, mi, :] = np.where(ktok <= qtok, 0.0, NEG)

        hid_own = np.ascontiguousarray(hs[own].T)      # [5120, 256]
        hidp = hid_own.reshape(HT, 128, OWN).transpose(1, 0, 2) \
            .reshape(128, HT * OWN).astype(bf16)

        wbn_c = wkvbn[:, 4 * DN * c:4 * DN * (c + 1)]  # [512, 512]
        wbn = wbn_c.reshape(KVT, 128, 4 * DN).transpose(1, 0, 2) \
            .reshape(128, KVT * 4 * DN).astype(bf16)

        in_maps.append({
            "hidp": hidp,
            "cosT": np.ascontiguousarray(cosT[:, own]),
            "sinTs": np.ascontiguousarray(sinTs[:, own]),
            "wqa_sh": WQA_s[c],
            "wkva_sh": WKVA_s[c],
            "wqb_sh": WQB_s[c],
            "wbv_sh": WBV_s[c],
            "wo_sh": WO_s[c],
            "wbn": wbn,
            "mask4": mask4,
            "ident": ident, "ones128": ones128, "onesrow": onesrow,
        })
    return in_maps


_NC_CACHE = None


def _get_nc():
    global _NC_CACHE
    if _NC_CACHE is None:
        _NC_CACHE = build_bass()
    return _NC_CACHE


def run(inputs, trace=False):
    nc = _get_nc()
    in_maps = _host_prep(inputs)
    res = run_bass_kernel_spmd(nc, in_maps, list(range(NCORES)), trace=trace)
    out = np.empty((T, HID), np.float32)
    for c in range(NCORES):
        oT = res.results[c]["outT"]
        out[128 * c:128 * (c + 1)] = oT[:, 0:128].T
        out[128 * (15 - c):128 * (16 - c)] = oT[:, 128:256].T
    return out, res


def kernel(**inputs):
    out, _ = run(inputs, trace=False)
    return out


# revision 15
# speedup vs baseline: 1.9801x; 1.0093x over previous
"""DeepseekV2 MLA prefill attention on 8 NeuronCores (Trainium2, Bass/Tile).

Sharding: token-parallel attention with zigzag blocks (core c owns token
blocks {c, 15-c}); all large weights are uploaded row-sharded (1/8 per core)
and broadcast on-device via AllGather into internal DRAM, so every weight
byte crosses the host link exactly once. Weight tensors are pre-swizzled on
host so each SBUF slab load is partition-contiguous (KB-sized descriptors).

Attention softmax uses a constant exp bias (scores are bounded; verified
max |scale*s| ~ 5.5 << 16) so exp reads score PSUM directly -- no row-max
pass, no second exp pass. Normalization is folded into the P^T transpose by
multiplying against diag(1/rowsum) instead of the identity. P@V accumulates
across units in PSUM.

SPMD: one program for all cores; per-core variation carried by mask data.
"""
import sys
import json

sys.path.insert(0, "/opt/trn_rl_repo")

import numpy as np
import ml_dtypes

import concourse.bass as bass
import concourse.mybir as mybir
import concourse.tile as tile
from concourse.bass_utils import run_bass_kernel_spmd

F32 = mybir.dt.float32
F32R = mybir.dt.float32r
BF16 = mybir.dt.bfloat16

T = 2048
H = 32
HID = 5120
QL = 1536
KVL = 512
DN = 128
DR = 64
DQK = DN + DR
DV = 128
EPS = 1e-6
SCALING = DQK ** -0.5
NCORES = 8
OWN = 256
CH = 512
NEG = -1e30
EB = 16.0            # constant exp bias: exp(SCALING*s - EB)

HT = HID // 128      # 40
QLT = QL // 128      # 12
KVT = KVL // 128     # 4
KVT5 = KVT + 1       # 4 latent slabs + 1 (padded) rope slab

# attention units: u0-u2 = B-tile keys [0:512/512:1024/1024:1536],
# u3 = A-tile keys [0:512], u4 = A-tile keys [512:1024] (active c>=4),
# u5 = B-tile keys [1536:2048] (active c<4). Inactive units fully masked.
UQS = [1, 1, 1, 0, 0, 1]             # 1 = B tile (q cols 128:256)
UKO = [0, CH, 2 * CH, 0, CH, 3 * CH]
MASKED = [2, 3, 4, 5]                # units with an additive mask
AUNITS = (3, 4)
BUNITS = (0, 1, 2, 5)
# vh slot lists per unit (vfull rows are rank-major: slot k=2r+s ~ block
# b = r (s=0) or 15-r (s=1); token block b -> slot 2b if b<8 else 2(15-b)+1)
U_V = [[0, 2, 4, 6], [8, 10, 12, 14], [15, 13, 11, 9],
       [0, 2, 4, 6], [8, 10, 12, 14], [7, 5, 3, 1]]


def legalize_sync_waits(nc):
    """This container's walrus accepts at most one sync-wait per instruction;
    split extras onto standalone EventSemaphore waits just before (same
    engine; engine streams preserve intra-block order)."""
    m = json.loads(nc.to_json_bytes())
    ctr = [0]

    def fresh():
        ctr[0] += 1
        return f"I-lw-{ctr[0]}"

    for f in m["functions"]:
        for bb in f["blocks"]:
            out = []
            for ins in bb["instructions"]:
                si = ins.get("sync_info")
                waits = (si or {}).get("on_wait") or []
                if len(waits) > 1:
                    for w in waits[:-1]:
                        out.append({
                            "debug": ins.get("debug", 0),
                            "engine": ins["engine"],
                            "ins": [], "outs": [],
                            "name": fresh(),
                            "opcode": "EventSemaphore",
                            "sync_info": {"on_update": [], "on_wait": [w]},
                        })
                    si["on_wait"] = waits[-1:]
                out.append(ins)
            bb["instructions"] = out
    nc.m = mybir.module_from_json_bytes(json.dumps(m).encode())
    return nc


def build_bass():
    nc = bass.Bass()
    AL = mybir.AluOpType
    AF = mybir.ActivationFunctionType

    dp = nc.declare_dram_parameter
    hidp_d = dp("hidp", [128, HT * OWN], BF16, isOutput=False)
    cosT_d = dp("cosT", [DR, OWN], F32, isOutput=False)
    sinTs_d = dp("sinTs", [DR, OWN], F32, isOutput=False)
    wqa_d = dp("wqa_sh", [QL // 8, HID], BF16, isOutput=False)
    wkva_d = dp("wkva_sh", [KVT5 * 128 // 8, HID], BF16, isOutput=False)
    wqb_d = dp("wqb_sh", [H * 128 // 8, QLT * DQK], BF16, isOutput=False)
    wbv_d = dp("wbv_sh", [KVL // 8, H * DV], BF16, isOutput=False)
    wo_d = dp("wo_sh", [HID // 8, H * DV], BF16, isOutput=False)
    wbn_d = dp("wbn", [128, KVT * 4 * DN], BF16, isOutput=False)
    mask4_d = dp("mask4", [128, 4, CH], F32, isOutput=False)
    ident_d = dp("ident", [128, 128], BF16, isOutput=False)
    ones128_d = dp("ones128", [128, 1], F32R, isOutput=False)
    onesrow_d = dp("onesrow", [1, 128], F32, isOutput=False)
    outT_d = dp("outT", [HID, OWN], F32, isOutput=True)

    RG = [list(range(NCORES))]

    with tile.TileContext(nc) as tc:
        from contextlib import ExitStack
        st = ExitStack()
        const = st.enter_context(tc.tile_pool(name="const", bufs=1))
        dram = st.enter_context(tc.tile_pool(name="dram", bufs=1, space="DRAM"))

        # ---- AG destinations (internal DRAM, Shared) ----
        gwqa = dram.tile([QL, HID], BF16, addr_space="Shared")
        gwkva = dram.tile([KVT5 * 128, HID], BF16, addr_space="Shared")
        gwqb = dram.tile([H * 128, QLT * DQK], BF16, addr_space="Shared")
        gwbv = dram.tile([KVL, H * DV], BF16, addr_space="Shared")
        gwo = dram.tile([HID, H * DV], BF16, addr_space="Shared")
        agin = dram.tile([KVL + DR, OWN], BF16)
        agkv = dram.tile([NCORES * (KVL + DR), OWN], BF16, addr_space="Shared")
        vshard = dram.tile([OWN, H * DV], BF16)
        vfull = dram.tile([T, H * DV], BF16, addr_space="Shared")
        ktshard = dram.tile([4 * DN, T], BF16)
        ktfull = dram.tile([H * DN, T], BF16, addr_space="Shared")

        def ag(inp, outp):
            nc.gpsimd.collective_compute(
                "AllGather", AL.bypass, replica_groups=RG,
                ins=[inp.opt()], outs=[outp.opt()])

        # weight broadcasts: stage each shard param into internal DRAM
        # (collectives cannot read IO tensors), then AllGather. Ordered by
        # first use so the CC queue never blocks a consumer longer than
        # needed.
        swqa = dram.tile([QL // 8, HID], BF16)
        swkva = dram.tile([KVT5 * 128 // 8, HID], BF16)
        swqb = dram.tile([H * 128 // 8, QLT * DQK], BF16)
        swbv = dram.tile([KVL // 8, H * DV], BF16)
        swo = dram.tile([HID // 8, H * DV], BF16)
        nc.sync.dma_start(swkva[:], wkva_d[:])
        nc.sync.dma_start(swqa[:], wqa_d[:])
        nc.sync.dma_start(swbv[:], wbv_d[:])
        nc.sync.dma_start(swqb[:], wqb_d[:])
        nc.sync.dma_start(swo[1:HID // 8, :], wo_d[1:HID // 8, :])
        ag(swkva[:], gwkva[:])
        ag(swqa[:], gwqa[:])
        ag(swbv[:], gwbv[:])
        ag(swqb[:], gwqb[:])

        # ---- constants ----
        ident = const.tile([128, 128], BF16)
        nc.sync.dma_start(ident[:], ident_d[:])
        ones128 = const.tile([128, 1], F32R)
        nc.sync.dma_start(ones128[:], ones128_d[:])
        onesrow = const.tile([1, 128], F32)
        nc.sync.dma_start(onesrow[:], onesrow_d[:])
        mask4 = const.tile([128, 4, CH], F32)
        nc.sync.dma_start(mask4[:], mask4_d[:])
        cosT = const.tile([128, OWN], F32)
        nc.sync.dma_start(cosT[0:DR, :], cosT_d[:])
        nc.sync.dma_start(cosT[64:64 + DR, :], cosT_d[:])
        sinTs = const.tile([128, OWN], F32)
        nc.sync.dma_start(sinTs[0:DR, :], sinTs_d[:])
        nc.sync.dma_start(sinTs[64:64 + DR, :], sinTs_d[:])
        epsc = const.tile([1, 1], F32)
        nc.vector.memset(epsc[:], EPS)
        ebias = const.tile([128, 1], F32)
        nc.vector.memset(ebias[:], -EB)

        # =========== phase B: down projections (transposed) ===========
        latp = st.enter_context(tc.tile_pool(name="latp", bufs=1))
        ph = ExitStack()
        hidp = ph.enter_context(tc.tile_pool(name="hidp", bufs=1))
        wsl = ph.enter_context(tc.tile_pool(name="wsl", bufs=2))
        rawp = ph.enter_context(tc.tile_pool(name="rawp", bufs=1))
        psB = ph.enter_context(tc.tile_pool(name="psB", bufs=4, space="PSUM"))
        psS = ph.enter_context(tc.tile_pool(name="psS", bufs=2, space="PSUM"))

        hidT = hidp.tile([128, HT, OWN], BF16)
        nc.sync.dma_start(hidT[:], hidp_d.rearrange("p (a t) -> p a t", a=HT))

        latq = rawp.tile([128, QLT, OWN], F32)
        latkv = rawp.tile([128, KVT5, OWN], F32)

        for lt in range(KVT5):
            wslab = wsl.tile([128, HT, 128], BF16, tag="wslab")
            nc.sync.dma_start(
                wslab[:], gwkva[128 * lt:128 * (lt + 1), :]
                .rearrange("p (a c) -> p a c", a=HT))
            ps = psB.tile([128, OWN], F32, tag="dps")
            for ht in range(HT):
                nc.tensor.matmul(ps[:], wslab[:, ht, :], hidT[:, ht, :],
                                 start=(ht == 0), stop=(ht == HT - 1))
            nc.scalar.copy(latkv[:, lt, :], ps[:])

        for lt in range(QLT):
            wslab = wsl.tile([128, HT, 128], BF16, tag="wslab")
            nc.sync.dma_start(
                wslab[:], gwqa[128 * lt:128 * (lt + 1), :]
                .rearrange("p (a c) -> p a c", a=HT))
            ps = psB.tile([128, OWN], F32, tag="dps")
            for ht in range(HT):
                nc.tensor.matmul(ps[:], wslab[:, ht, :], hidT[:, ht, :],
                                 start=(ht == 0), stop=(ht == HT - 1))
            nc.scalar.copy(latq[:, lt, :], ps[:])

        # ---- rmsnorm factors via squares + ones-matmul ----
        latq_n = latp.tile([128, QLT, OWN], BF16)
        latkv_n = latp.tile([128, KVT, OWN], BF16)

        def rmsnorm(lat, lat_n, nt, L):
            ssq = psS.tile([1, OWN], F32, tag="ssq")
            for lt in range(nt):
                sq = rawp.tile([128, OWN], F32R, tag="sqscratch", bufs=2)
                nc.vector.tensor_tensor(out=sq[:], in0=lat[:, lt, :],
                                        in1=lat[:, lt, :], op=AL.mult)
                nc.tensor.matmul(ssq[:], ones128[:], sq[:],
                                 start=(lt == 0), stop=(lt == nt - 1))
            f = rawp.tile([1, OWN], F32, tag="fscratch", bufs=2)
            nc.scalar.activation(f[:], ssq[:], AF.Sqrt, bias=epsc[:],
                                 scale=1.0 / L)
            fr = rawp.tile([1, OWN], F32, tag="frscratch", bufs=2)
            nc.vector.reciprocal(fr[:], f[:])
            fb = psS.tile([128, OWN], F32, tag="fbcast")
            nc.tensor.matmul(fb[:], onesrow[:], fr[:], start=True, stop=True)
            for lt in range(nt):
                nc.vector.tensor_tensor(out=lat_n[:, lt, :], in0=lat[:, lt, :],
                                        in1=fb[:], op=AL.mult)

        rmsnorm(latkv, latkv_n, KVT, KVL)
        rmsnorm(latq, latq_n, QLT, QL)

        # ---- rope k_pe (deinterleave folded into wkva on host) ----
        kpsw = rawp.tile([128, OWN], F32)
        nc.sync.dma_start(kpsw[0:32, :], latkv[32:64, KVT, :])
        nc.sync.dma_start(kpsw[32:64, :], latkv[0:32, KVT, :])
        kpc = rawp.tile([128, OWN], F32)
        nc.vector.tensor_tensor(out=kpc[0:DR, :], in0=latkv[0:DR, KVT, :],
                                in1=cosT[0:DR, :], op=AL.mult)
        nc.vector.tensor_tensor(out=kpsw[0:DR, :], in0=kpsw[0:DR, :],
                                in1=sinTs[0:DR, :], op=AL.mult)
        kpeR = rawp.tile([128, OWN], BF16)
        nc.vector.tensor_tensor(out=kpeR[0:DR, :], in0=kpc[0:DR, :],
                                in1=kpsw[0:DR, :], op=AL.add)

        # assemble AG input: rows 0:512 normalized latent, 512:576 roped kpe
        for lt in range(KVT):
            nc.scalar.dma_start(agin[128 * lt:128 * (lt + 1), :],
                                latkv_n[:, lt, :])
        nc.scalar.dma_start(agin[KVL:KVL + DR, :], kpeR[0:DR, :])
        ag(agin[:], agkv[:])
        ph.close()

        # =========== phase D: V (own tokens, all heads) -> AG ===========
        ph = ExitStack()
        wv = ph.enter_context(tc.tile_pool(name="wv", bufs=2))
        psD = ph.enter_context(tc.tile_pool(name="psD", bufs=2, space="PSUM"))
        evp = ph.enter_context(tc.tile_pool(name="evp", bufs=3))

        for vc in range(8):             # 8 chunks of 512 v-columns
            wvs = wv.tile([128, KVT, CH], BF16, tag="wvs")
            nc.sync.dma_start(
                wvs[:], gwbv[:, CH * vc:CH * (vc + 1)]
                .rearrange("(l p) c -> p l c", p=128))
            for tt in range(2):         # 2 token tiles of 128
                ps = psD.tile([128, CH], F32, tag="vps")
                for lt in range(KVT):
                    nc.tensor.matmul(
                        ps[:], latkv_n[:, lt, 128 * tt:128 * (tt + 1)],
                        wvs[:, lt, :], start=(lt == 0), stop=(lt == KVT - 1))
                ev = evp.tile([128, CH], BF16, tag="vev")
                nc.scalar.copy(ev[:], ps[:])
                nc.scalar.dma_start(
                    vshard[128 * tt:128 * (tt + 1), CH * vc:CH * (vc + 1)],
                    ev[:])

        # vfull AG issued after ktfull (phase E) so kt is gathered first

        # =========== phase E: K^T (this core's 4 heads, all tokens) -> AG ====
        wkn = ph.enter_context(tc.tile_pool(name="wkn", bufs=1))
        wkns = wkn.tile([128, KVT, 4 * DN], BF16)
        nc.sync.dma_start(wkns[:], wbn_d.rearrange("p (l c) -> p l c", l=KVT))

        agp = ph.enter_context(tc.tile_pool(name="agp", bufs=2))
        for r in range(NCORES):
            slab = agp.tile([128, KVT, OWN], BF16, tag="agslab")
            nc.sync.dma_start(
                slab[:], agkv[(KVL + DR) * r:(KVL + DR) * r + KVL, :]
                .rearrange("(l p) t -> p l t", p=128))
            for hl in range(4):
                ps = psD.tile([128, OWN], F32, tag="ktps")
                for lt in range(KVT):
                    nc.tensor.matmul(ps[:], wkns[:, lt, DN * hl:DN * (hl + 1)],
                                     slab[:, lt, :],
                                     start=(lt == 0), stop=(lt == KVT - 1))
                ev = evp.tile([128, OWN], BF16, tag="ktev")
                nc.scalar.copy(ev[:], ps[:])
                # token-ordered columns: chunk r covers blocks r and 15-r
                nc.scalar.dma_start(
                    ktshard[DN * hl:DN * (hl + 1), 128 * r:128 * (r + 1)],
                    ev[:, 0:128])
                nc.scalar.dma_start(
                    ktshard[DN * hl:DN * (hl + 1),
                            128 * (15 - r):128 * (16 - r)],
                    ev[:, 128:256])

        ag(ktshard[:], ktfull[:])
        vb = agp.tile([1, 128], BF16, tag="agslab")
        nc.sync.dma_start(vb[:], vshard[0:1, 0:128])
        nc.sync.dma_start(vshard[0:1, 0:128], vb[:])
        ag(vshard[:], vfull[:])
        wob = agp.tile([1, H * DV], BF16, tag="agslab")
        nc.sync.dma_start(wob[:], wo_d[0:1, :])
        nc.sync.dma_start(swo[0:1, :], wob[:])
        ag(swo[:], gwo[:])

        # k_pe^T assembly (token-ordered, shared across heads)
        kpeT = const.tile([128, T], BF16)
        for b in range(16):
            rb = min(b, 15 - b)
            colsl = slice(0, 128) if b < 8 else slice(128, 256)
            src_ap = agkv[(KVL + DR) * rb + KVL:(KVL + DR) * rb + KVL + DR,
                          colsl]
            nc.sync.dma_start(kpeT[0:DR, 128 * b:128 * (b + 1)], src_ap)
            nc.sync.dma_start(kpeT[64:64 + DR, 128 * b:128 * (b + 1)], src_ap)
        ph.close()

        # =========== phase F: Q up-projection + rope (all heads) ===========
        qp_pool = st.enter_context(tc.tile_pool(name="qp", bufs=1))
        qTn = qp_pool.tile([128, H, OWN], BF16)
        qTp = qp_pool.tile([128, H // 2, OWN], BF16)
        attnT = qp_pool.tile([128, H, OWN], F32R)

        ph = ExitStack()
        wqb = ph.enter_context(tc.tile_pool(name="wqb", bufs=2))
        psF = ph.enter_context(tc.tile_pool(name="psF", bufs=3, space="PSUM"))
        rp = ph.enter_context(tc.tile_pool(name="rp", bufs=3))

        for h in range(H):
            ws = wqb.tile([128, QLT, DQK], BF16, tag="wqbs")
            nc.sync.dma_start(
                ws[:], gwqb[128 * h:128 * (h + 1), :]
                .rearrange("p (l c) -> p l c", l=QLT))
            pb = 0 if h < 16 else 64
            hs_ = h % 16
            psn = psF.tile([128, OWN], F32, tag="qnps")
            psp = psF.tile([128, OWN], F32, tag="qpps")
            for lt in range(QLT):
                nc.tensor.matmul(psn[:], ws[:, lt, 0:DN], latq_n[:, lt, :],
                                 start=(lt == 0), stop=(lt == QLT - 1))
            for lt in range(QLT):
                nc.tensor.matmul(psp[0:DR, :], ws[:, lt, DN:DQK],
                                 latq_n[:, lt, :],
                                 start=(lt == 0), stop=(lt == QLT - 1))
            nc.scalar.copy(qTn[:, h, :], psn[:])
            praw = rp.tile([128, OWN], F32, tag="praw")
            nc.scalar.copy(praw[0:DR, :], psp[0:DR, :])
            psw = rp.tile([128, OWN], F32, tag="psw")
            nc.sync.dma_start(psw[0:32, :], praw[32:DR, :])
            nc.sync.dma_start(psw[32:DR, :], praw[0:32, :])
            pc = rp.tile([128, OWN], F32, tag="pc")
            nc.vector.tensor_tensor(out=pc[0:DR, :], in0=praw[0:DR, :],
                                    in1=cosT[0:DR, :], op=AL.mult)
            nc.vector.tensor_tensor(out=psw[0:DR, :], in0=psw[0:DR, :],
                                    in1=sinTs[0:DR, :], op=AL.mult)
            if pb == 0:
                nc.vector.tensor_tensor(out=qTp[0:DR, hs_, :], in0=pc[0:DR, :],
                                        in1=psw[0:DR, :], op=AL.add)
            else:
                rshift = rp.tile([128, OWN], BF16, tag="rshift")
                nc.vector.tensor_tensor(out=rshift[0:DR, :], in0=pc[0:DR, :],
                                        in1=psw[0:DR, :], op=AL.add)
                nc.sync.dma_start(qTp[pb:pb + DR, hs_, :], rshift[0:DR, :])
        ph.close()

        # =========== phase G: attention ===========
        ph = ExitStack()
        ap = ph.enter_context(tc.tile_pool(name="ap", bufs=2))
        sp_pool = ph.enter_context(tc.tile_pool(name="spp", bufs=2))
        stt = ph.enter_context(tc.tile_pool(name="stt", bufs=2))
        psG = ph.enter_context(tc.tile_pool(name="psG", bufs=2, space="PSUM"))
        psT = ph.enter_context(tc.tile_pool(name="psT", bufs=4, space="PSUM"))
        psV = ph.enter_context(tc.tile_pool(name="psV", bufs=2, space="PSUM"))

        for h in range(H):
            kt = ap.tile([128, T], BF16, tag="kt")
            nc.sync.dma_start(kt[:], ktfull[DN * h:DN * (h + 1), :])
            vh = ap.tile([128, 16, DV], BF16, tag="vh")
            nc.sync.dma_start(
                vh[:], vfull.rearrange("(k p) c -> p k c", p=128)
                [:, :, DV * h:DV * (h + 1)])

            pb = 0 if h < 16 else 64
            hs_ = h % 16

            # scores per unit: nope+rope MMs, mask (in PSUM), exp -> escr + sum
            Pb = sp_pool.tile([128, 6, CH], BF16, tag="Pb")
            sumu = stt.tile([128, 6], F32, tag="sumu")
            escrs = []
            for u in (0, 1, 2, 5, 3, 4):
                q0 = 128 * UQS[u]
                ps = psG.tile([128, CH], F32, tag="sps", name=f"s{h}_{u}")
                nc.tensor.matmul(ps[:], qTn[:, h, q0:q0 + 128],
                                 kt[:, UKO[u]:UKO[u] + CH],
                                 start=True, stop=False)
                nc.tensor.matmul(ps[:], qTp[pb:pb + DR, hs_, q0:q0 + 128],
                                 kpeT[pb:pb + DR, UKO[u]:UKO[u] + CH],
                                 start=False, stop=True)
                if u in MASKED:
                    mi = MASKED.index(u)
                    nc.vector.tensor_tensor(out=ps[:], in0=ps[:],
                                            in1=mask4[:, mi, :], op=AL.add)
                escr = sp_pool.tile([128, CH], F32, tag="escr", bufs=8,
                                    name=f"e{h}_{u}")
                nc.scalar.activation(escr[:], ps[:], AF.Exp,
                                     bias=ebias[:], scale=SCALING,
                                     accum_out=sumu[:, u:u + 1])
                escrs.append(escr)

            # group sums -> diag(1/sum) matrices
            sB0 = stt.tile([128, 1], F32, tag="sB0")
            nc.vector.tensor_tensor(out=sB0[:], in0=sumu[:, 0:1],
                                    in1=sumu[:, 1:2], op=AL.add)
            sB1 = stt.tile([128, 1], F32, tag="sB1")
            nc.vector.tensor_tensor(out=sB1[:], in0=sumu[:, 2:3],
                                    in1=sumu[:, 5:6], op=AL.add)
            sB = stt.tile([128, 1], F32, tag="sB")
            nc.vector.tensor_tensor(out=sB[:], in0=sB0[:], in1=sB1[:],
                                    op=AL.add)
            sA = stt.tile([128, 1], F32, tag="sA")
            nc.vector.tensor_tensor(out=sA[:], in0=sumu[:, 3:4],
                                    in1=sumu[:, 4:5], op=AL.add)
            rA = stt.tile([128, 1], F32, tag="rA")
            nc.vector.reciprocal(rA[:], sA[:])
            rB = stt.tile([128, 1], F32, tag="rB")
            nc.vector.reciprocal(rB[:], sB[:])

            # normalize during the f32->bf16 cast (q is on partitions here)
            for i, u in enumerate((0, 1, 2, 5, 3, 4)):
                r_ = rA if u in AUNITS else rB
                if i % 3 == 2:
                    nc.scalar.mul(Pb[:, u, :], escrs[i][:], r_[:])
                else:
                    nc.vector.tensor_scalar_mul(Pb[:, u, :], escrs[i][:],
                                                r_[:])

            # P^T + PV accumulated in PSUM
            psVB = psV.tile([128, DV], F32, tag="pv", name=f"pvB{h}")
            psVA = psV.tile([128, DV], F32, tag="pv", name=f"pvA{h}")
            nmm = 0
            for u in range(6):
                isA = u in AUNITS
                dst = psVA if isA else psVB
                first = (u == 3 and True) if isA else (u == 0)
                for kb in range(4):
                    tp = psT.tile([128, 128], BF16, tag="tp")
                    nc.tensor.transpose(tp[:], Pb[:, u, 128 * kb:128 * (kb + 1)],
                                        ident[:])
                    ptT = stt.tile([128, 128], BF16, tag="ptT", bufs=4)
                    if nmm % 3 == 2:
                        nc.scalar.copy(ptT[:], tp[:])
                    else:
                        nc.vector.tensor_copy(ptT[:], tp[:])
                    nmm += 1
                    nc.tensor.matmul(
                        dst[:], vh[:, U_V[u][kb], :], ptT[:],
                        start=(first and kb == 0),
                        stop=((u == 4 and kb == 3) if isA
                              else (u == 5 and kb == 3)))
            nc.scalar.copy(attnT[:, h, 0:128], psVA[:])
            nc.vector.tensor_copy(attnT[:, h, 128:256], psVB[:])
        ph.close()

        # =========== phase H: out projection ===========
        ph = ExitStack()
        wop = ph.enter_context(tc.tile_pool(name="wop", bufs=2))
        psH = ph.enter_context(tc.tile_pool(name="psH", bufs=4, space="PSUM"))
        oev = ph.enter_context(tc.tile_pool(name="oev", bufs=3))
        abf_p = ph.enter_context(tc.tile_pool(name="abf", bufs=1))
        attnB = abf_p.tile([128, H, OWN], BF16)
        for ct in range(H):
            nc.vector.tensor_copy(attnB[:, ct, :], attnT[:, ct, :])
        for oc in range(HID // 128):
            ws = wop.tile([128, H, 128], BF16, tag="wos")
            nc.sync.dma_start(
                ws[:], gwo[128 * oc:128 * (oc + 1), :]
                .rearrange("p (t c) -> p t c", t=H))
            ps = psH.tile([128, OWN], F32, tag="ops")
            for ct in range(H):
                nc.tensor.matmul(ps[:], ws[:, ct, :], attnB[:, ct, :],
                                 start=(ct == 0), stop=(ct == H - 1))
            ev = oev.tile([128, OWN], F32, tag="oev")
            nc.scalar.copy(ev[:], ps[:])
            nc.sync.dma_start(outT_d[128 * oc:128 * (oc + 1), :], ev[:])
        ph.close()
        st.close()

    nc.finalize()
    legalize_sync_waits(nc)
    return nc


_DEINT = np.array([2 * r if r < 32 else 2 * r - 63 for r in range(DR)])


def _host_prep(inputs):
    f32 = np.float32
    bf16 = ml_dtypes.bfloat16
    hs = np.asarray(inputs["hidden_states"], f32)
    cos = np.asarray(inputs["cos"], f32).reshape(T, DR)
    sin = np.asarray(inputs["sin"], f32).reshape(T, DR)
    wq_a = np.asarray(inputs["wq_a"], f32)
    q_ln = np.asarray(inputs["q_a_ln_w"], f32)
    wq_b = np.asarray(inputs["wq_b"], f32)
    wkv_a = np.asarray(inputs["wkv_a"], f32)
    kv_ln = np.asarray(inputs["kv_a_ln_w"], f32)
    wkv_b = np.asarray(inputs["wkv_b"], f32)
    wo = np.asarray(inputs["wo"], f32)

    # fold ln weights into up-projections
    wq_b = wq_b * q_ln[:, None]
    wkv_b = wkv_b * kv_ln[:, None]

    # deinterleave fold: q_pe columns of wq_b, k_pe columns of wkv_a
    wqbp = wq_b.copy()
    for h in range(H):
        pe = wq_b[:, h * DQK + DN:h * DQK + DQK]
        wqbp[:, h * DQK + DN:h * DQK + DQK] = pe[:, _DEINT]
    wkvap = np.zeros((HID, KVT5 * 128), f32)
    wkvap[:, :KVL] = wkv_a[:, :KVL]
    wkvap[:, KVL:KVL + DR] = wkv_a[:, KVL:][:, _DEINT]

    # split wkv_b into nope / v column groups (head-major)
    wkvbn = np.concatenate(
        [wkv_b[:, h * 256:h * 256 + DN] for h in range(H)], axis=1)
    wkvbv = np.concatenate(
        [wkv_b[:, h * 256 + DN:h * 256 + 256] for h in range(H)], axis=1)

    # swizzles: slab-major, partition-contiguous layouts
    WQA = wq_a.reshape(HT, 128, QLT, 128).transpose(2, 1, 0, 3) \
        .reshape(QL, HID).astype(bf16)
    WKVA = wkvap.reshape(HT, 128, KVT5, 128).transpose(2, 1, 0, 3) \
        .reshape(KVT5 * 128, HID).astype(bf16)
    WQB = wqbp.reshape(QLT, 128, H, DQK).transpose(2, 1, 0, 3) \
        .reshape(H * 128, QLT * DQK).astype(bf16)
    WBV = wkvbv.astype(bf16)                      # [512, 4096] natural
    WO = wo.reshape(H, 128, HT, 128).transpose(2, 1, 0, 3) \
        .reshape(HID, H * DV).astype(bf16)

    cosT = np.ascontiguousarray(cos.T)
    sinT = np.ascontiguousarray(sin.T)
    sinTs = sinT.copy()
    sinTs[0:32] = -sinT[0:32]

    ident = np.eye(128, dtype=bf16)
    ones128 = np.ones((128, 1), f32)
    onesrow = np.ones((1, 128), f32)

    def shard(a):
        n = a.shape[0] // NCORES
        return [np.ascontiguousarray(a[c * n:(c + 1) * n]) for c in
                range(NCORES)]

    WQA_s, WKVA_s, WQB_s, WBV_s, WO_s = (shard(WQA), shard(WKVA), shard(WQB),
                                         shard(WBV), shard(WO))

    qr = np.arange(128)[:, None]
    kr = np.arange(CH)[None, :]

    in_maps = []
    for c in range(NCORES):
        bA, bB = c, 15 - c
        own = np.r_[np.arange(128 * bA, 128 * bA + 128),
                    np.arange(128 * bB, 128 * bB + 128)]
        # masks for units u2,u3,u4,u5
        mask4 = np.zeros((128, 4, CH), f32)
        specs = [(bB, 2 * CH, True),          # u2
                 (bA, 0, True),               # u3
                 (bA, CH, c >= 4),            # u4
                 (bB, 3 * CH, c < 4)]         # u5
        for mi, (qb, koff, active) in enumerate(specs):
            if not active:
                mask4[:, mi, :] = NEG
            else:
                qtok = 128 * qb + qr
                ktok = koff + kr
                mask4[:, mi, :] = np.where(ktok <= qtok, 0.0, NEG)

        hid_own = np.ascontiguousarray(hs[own].T)      # [5120, 256]
        hidp = hid_own.reshape(HT, 128, OWN).transpose(1, 0, 2) \
            .reshape(128, HT * OWN).astype(bf16)

        wbn_c = wkvbn[:, 4 * DN * c:4 * DN * (c + 1)]  # [512, 512]
        wbn = wbn_c.reshape(KVT, 128, 4 * DN).transpose(1, 0, 2) \
            .reshape(128, KVT * 4 * DN).astype(bf16)

        in_maps.append({
            "hidp": hidp,
            "cosT": np.ascontiguousarray(cosT[:, own]),
            "sinTs": np.ascontiguousarray(sinTs[:, own]),
            "wqa_sh": WQA_s[c],
            "wkva_sh": WKVA_s[c],
            "wqb_sh": WQB_s[c],
            "wbv_sh": WBV_s[c],
            "wo_sh": WO_s[c],
            "wbn": wbn,
            "mask4": mask4,
            "ident": ident, "ones128": ones128, "onesrow": onesrow,
        })
    return in_maps


_NC_CACHE = None


def _get_nc():
    global _NC_CACHE
    if _NC_CACHE is None:
        _NC_CACHE = build_bass()
    return _NC_CACHE


def run(inputs, trace=False):
    nc = _get_nc()
    in_maps = _host_prep(inputs)
    res = run_bass_kernel_spmd(nc, in_maps, list(range(NCORES)), trace=trace)
    out = np.empty((T, HID), np.float32)
    for c in range(NCORES):
        oT = res.results[c]["outT"]
        out[128 * c:128 * (c + 1)] = oT[:, 0:128].T
        out[128 * (15 - c):128 * (16 - c)] = oT[:, 128:256].T
    return out, res


def kernel(**inputs):
    out, _ = run(inputs, trace=False)
    return out


# revision 17
# speedup vs baseline: 2.0140x; 1.0171x over previous
"""DeepseekV2 MLA prefill attention on 8 NeuronCores (Trainium2, Bass/Tile).

Sharding: token-parallel attention with zigzag blocks (core c owns token
blocks {c, 15-c}); all large weights are uploaded row-sharded (1/8 per core)
and broadcast on-device via AllGather into internal DRAM, so every weight
byte crosses the host link exactly once. Weight tensors are pre-swizzled on
host so each SBUF slab load is partition-contiguous (KB-sized descriptors).

Attention softmax uses a constant exp bias (scores are bounded; verified
max |scale*s| ~ 5.5 << 16) so exp reads score PSUM directly -- no row-max
pass, no second exp pass. Normalization is folded into the P^T transpose by
multiplying against diag(1/rowsum) instead of the identity. P@V accumulates
across units in PSUM.

SPMD: one program for all cores; per-core variation carried by mask data.
"""
import sys
import json

sys.path.insert(0, "/opt/trn_rl_repo")

import numpy as np
import ml_dtypes

import concourse.bass as bass
import concourse.mybir as mybir
import concourse.tile as tile
from concourse.bass_utils import run_bass_kernel_spmd

F32 = mybir.dt.float32
F32R = mybir.dt.float32r
BF16 = mybir.dt.bfloat16

T = 2048
H = 32
HID = 5120
QL = 1536
KVL = 512
DN = 128
DR = 64
DQK = DN + DR
DV = 128
EPS = 1e-6
SCALING = DQK ** -0.5
NCORES = 8
OWN = 256
CH = 512
NEG = -1e30
EB = 16.0            # constant exp bias: exp(SCALING*s - EB)

HT = HID // 128      # 40
QLT = QL // 128      # 12
KVT = KVL // 128     # 4
KVT5 = KVT + 1       # 4 latent slabs + 1 (padded) rope slab

# attention units: u0-u2 = B-tile keys [0:512/512:1024/1024:1536],
# u3 = A-tile keys [0:512], u4 = A-tile keys [512:1024] (active c>=4),
# u5 = B-tile keys [1536:2048] (active c<4). Inactive units fully masked.
UQS = [1, 1, 1, 0, 0, 1]             # 1 = B tile (q cols 128:256)
UKO = [0, CH, 2 * CH, 0, CH, 3 * CH]
MASKED = [2, 3, 4, 5]                # units with an additive mask
AUNITS = (3, 4)
BUNITS = (0, 1, 2, 5)
# vh slot lists per unit (vfull rows are rank-major: slot k=2r+s ~ block
# b = r (s=0) or 15-r (s=1); token block b -> slot 2b if b<8 else 2(15-b)+1)
U_V = [[0, 2, 4, 6], [8, 10, 12, 14], [15, 13, 11, 9],
       [0, 2, 4, 6], [8, 10, 12, 14], [7, 5, 3, 1]]


def legalize_sync_waits(nc):
    """This container's walrus accepts at most one sync-wait per instruction;
    split extras onto standalone EventSemaphore waits just before (same
    engine; engine streams preserve intra-block order)."""
    m = json.loads(nc.to_json_bytes())
    ctr = [0]

    def fresh():
        ctr[0] += 1
        return f"I-lw-{ctr[0]}"

    for f in m["functions"]:
        for bb in f["blocks"]:
            out = []
            for ins in bb["instructions"]:
                si = ins.get("sync_info")
                waits = (si or {}).get("on_wait") or []
                if len(waits) > 1:
                    for w in waits[:-1]:
                        out.append({
                            "debug": ins.get("debug", 0),
                            "engine": ins["engine"],
                            "ins": [], "outs": [],
                            "name": fresh(),
                            "opcode": "EventSemaphore",
                            "sync_info": {"on_update": [], "on_wait": [w]},
                        })
                    si["on_wait"] = waits[-1:]
                out.append(ins)
            bb["instructions"] = out
    nc.m = mybir.module_from_json_bytes(json.dumps(m).encode())
    return nc


def build_bass():
    nc = bass.Bass()
    AL = mybir.AluOpType
    AF = mybir.ActivationFunctionType

    dp = nc.declare_dram_parameter
    hidp_d = dp("hidp", [128, HT * OWN], BF16, isOutput=False)
    cosT_d = dp("cosT", [DR, OWN], F32, isOutput=False)
    sinTs_d = dp("sinTs", [DR, OWN], F32, isOutput=False)
    wqa_d = dp("wqa_sh", [QL // 8, HID], BF16, isOutput=False)
    wkva_d = dp("wkva_sh", [KVT5 * 128 // 8, HID], BF16, isOutput=False)
    wqb_d = dp("wqb_sh", [H * 128 // 8, QLT * DQK], BF16, isOutput=False)
    wbv_d = dp("wbv_sh", [KVL // 8, H * DV], BF16, isOutput=False)
    wo_d = dp("wo_sh", [HID // 8, H * DV], BF16, isOutput=False)
    wbn_d = dp("wbn", [128, KVT * 4 * DN], BF16, isOutput=False)
    mask4_d = dp("mask4", [128, 4, CH], F32, isOutput=False)
    ident_d = dp("ident", [128, 128], BF16, isOutput=False)
    ones128_d = dp("ones128", [128, 1], F32R, isOutput=False)
    onesrow_d = dp("onesrow", [1, 128], F32, isOutput=False)
    outT_d = dp("outT", [HID, OWN], F32, isOutput=True)

    RG = [list(range(NCORES))]

    with tile.TileContext(nc) as tc:
        from contextlib import ExitStack
        st = ExitStack()
        const = st.enter_context(tc.tile_pool(name="const", bufs=1))
        dram = st.enter_context(tc.tile_pool(name="dram", bufs=1, space="DRAM"))

        # ---- AG destinations (internal DRAM, Shared) ----
        gwqa = dram.tile([QL, HID], BF16, addr_space="Shared")
        gwkva = dram.tile([KVT5 * 128, HID], BF16, addr_space="Shared")
        gwqb = dram.tile([H * 128, QLT * DQK], BF16, addr_space="Shared")
        gwbv = dram.tile([KVL, H * DV], BF16, addr_space="Shared")
        gwo = dram.tile([HID, H * DV], BF16, addr_space="Shared")
        agin = dram.tile([KVL + DR, OWN], BF16)
        agkv = dram.tile([NCORES * (KVL + DR), OWN], BF16, addr_space="Shared")
        vshard = dram.tile([OWN, H * DV], BF16)
        vfull = dram.tile([T, H * DV], BF16, addr_space="Shared")
        ktshard = dram.tile([4 * DN, T], BF16)
        ktfull = dram.tile([H * DN, T], BF16, addr_space="Shared")

        def ag(inp, outp):
            nc.gpsimd.collective_compute(
                "AllGather", AL.bypass, replica_groups=RG,
                ins=[inp.opt()], outs=[outp.opt()])

        # weight broadcasts: stage each shard param into internal DRAM
        # (collectives cannot read IO tensors), then AllGather. Ordered by
        # first use so the CC queue never blocks a consumer longer than
        # needed.
        swqa = dram.tile([QL // 8, HID], BF16)
        swkva = dram.tile([KVT5 * 128 // 8, HID], BF16)
        swqb = dram.tile([H * 128 // 8, QLT * DQK], BF16)
        swbv = dram.tile([KVL // 8, H * DV], BF16)
        swo = dram.tile([HID // 8, H * DV], BF16)
        nc.sync.dma_start(swkva[:], wkva_d[:])
        nc.sync.dma_start(swqa[:], wqa_d[:])
        nc.sync.dma_start(swbv[1:KVL // 8, :], wbv_d[1:KVL // 8, :])
        nc.sync.dma_start(swqb[1:H * 128 // 8, :], wqb_d[1:H * 128 // 8, :])
        nc.sync.dma_start(swo[1:HID // 8, :], wo_d[1:HID // 8, :])
        ag(swkva[:], gwkva[:])
        ag(swqa[:], gwqa[:])

        # ---- constants ----
        ident = const.tile([128, 128], BF16)
        nc.sync.dma_start(ident[:], ident_d[:])
        ones128 = const.tile([128, 1], F32R)
        nc.sync.dma_start(ones128[:], ones128_d[:])
        onesrow = const.tile([1, 128], F32)
        nc.sync.dma_start(onesrow[:], onesrow_d[:])
        mask4 = const.tile([128, 4, CH], F32)
        nc.sync.dma_start(mask4[:], mask4_d[:])
        cosT = const.tile([128, OWN], F32)
        nc.sync.dma_start(cosT[0:DR, :], cosT_d[:])
        nc.sync.dma_start(cosT[64:64 + DR, :], cosT_d[:])
        sinTs = const.tile([128, OWN], F32)
        nc.sync.dma_start(sinTs[0:DR, :], sinTs_d[:])
        nc.sync.dma_start(sinTs[64:64 + DR, :], sinTs_d[:])
        epsc = const.tile([1, 1], F32)
        nc.vector.memset(epsc[:], EPS)
        ebias = const.tile([128, 1], F32)
        nc.vector.memset(ebias[:], -EB)

        # =========== phase B: down projections (transposed) ===========
        latp = st.enter_context(tc.tile_pool(name="latp", bufs=1))
        ph = ExitStack()
        hidp = ph.enter_context(tc.tile_pool(name="hidp", bufs=1))
        wsl = ph.enter_context(tc.tile_pool(name="wsl", bufs=2))
        rawp = ph.enter_context(tc.tile_pool(name="rawp", bufs=1))
        psB = ph.enter_context(tc.tile_pool(name="psB", bufs=4, space="PSUM"))
        psS = ph.enter_context(tc.tile_pool(name="psS", bufs=2, space="PSUM"))

        hidT = hidp.tile([128, HT, OWN], BF16)
        nc.sync.dma_start(hidT[:], hidp_d.rearrange("p (a t) -> p a t", a=HT))

        latq = rawp.tile([128, QLT, OWN], F32)
        latkv = rawp.tile([128, KVT5, OWN], F32)

        for lt in range(KVT5):
            wslab = wsl.tile([128, HT, 128], BF16, tag="wslab")
            nc.sync.dma_start(
                wslab[:], gwkva[128 * lt:128 * (lt + 1), :]
                .rearrange("p (a c) -> p a c", a=HT))
            ps = psB.tile([128, OWN], F32, tag="dps")
            for ht in range(HT):
                nc.tensor.matmul(ps[:], wslab[:, ht, :], hidT[:, ht, :],
                                 start=(ht == 0), stop=(ht == HT - 1))
            nc.scalar.copy(latkv[:, lt, :], ps[:])

        for lt in range(QLT):
            wslab = wsl.tile([128, HT, 128], BF16, tag="wslab")
            nc.sync.dma_start(
                wslab[:], gwqa[128 * lt:128 * (lt + 1), :]
                .rearrange("p (a c) -> p a c", a=HT))
            ps = psB.tile([128, OWN], F32, tag="dps")
            for ht in range(HT):
                nc.tensor.matmul(ps[:], wslab[:, ht, :], hidT[:, ht, :],
                                 start=(ht == 0), stop=(ht == HT - 1))
            nc.scalar.copy(latq[:, lt, :], ps[:])

        # ---- rmsnorm factors via squares + ones-matmul ----
        latq_n = latp.tile([128, QLT, OWN], BF16)
        latkv_n = latp.tile([128, KVT, OWN], BF16)

        def rmsnorm(lat, lat_n, nt, L):
            ssq = psS.tile([1, OWN], F32, tag="ssq")
            for lt in range(nt):
                sq = rawp.tile([128, OWN], F32R, tag="sqscratch", bufs=2)
                nc.vector.tensor_tensor(out=sq[:], in0=lat[:, lt, :],
                                        in1=lat[:, lt, :], op=AL.mult)
                nc.tensor.matmul(ssq[:], ones128[:], sq[:],
                                 start=(lt == 0), stop=(lt == nt - 1))
            f = rawp.tile([1, OWN], F32, tag="fscratch", bufs=2)
            nc.scalar.activation(f[:], ssq[:], AF.Sqrt, bias=epsc[:],
                                 scale=1.0 / L)
            fr = rawp.tile([1, OWN], F32, tag="frscratch", bufs=2)
            nc.vector.reciprocal(fr[:], f[:])
            fb = psS.tile([128, OWN], F32, tag="fbcast")
            nc.tensor.matmul(fb[:], onesrow[:], fr[:], start=True, stop=True)
            for lt in range(nt):
                nc.vector.tensor_tensor(out=lat_n[:, lt, :], in0=lat[:, lt, :],
                                        in1=fb[:], op=AL.mult)

        rmsnorm(latkv, latkv_n, KVT, KVL)
        rmsnorm(latq, latq_n, QLT, QL)

        # ---- rope k_pe (deinterleave folded into wkva on host) ----
        kpsw = rawp.tile([128, OWN], F32)
        nc.sync.dma_start(kpsw[0:32, :], latkv[32:64, KVT, :])
        nc.sync.dma_start(kpsw[32:64, :], latkv[0:32, KVT, :])
        kpc = rawp.tile([128, OWN], F32)
        nc.vector.tensor_tensor(out=kpc[0:DR, :], in0=latkv[0:DR, KVT, :],
                                in1=cosT[0:DR, :], op=AL.mult)
        nc.vector.tensor_tensor(out=kpsw[0:DR, :], in0=kpsw[0:DR, :],
                                in1=sinTs[0:DR, :], op=AL.mult)
        kpeR = rawp.tile([128, OWN], BF16)
        nc.vector.tensor_tensor(out=kpeR[0:DR, :], in0=kpc[0:DR, :],
                                in1=kpsw[0:DR, :], op=AL.add)

        # assemble AG input: rows 0:512 normalized latent, 512:576 roped kpe
        for lt in range(KVT):
            nc.scalar.dma_start(agin[128 * lt:128 * (lt + 1), :],
                                latkv_n[:, lt, :])
        nc.scalar.dma_start(agin[KVL:KVL + DR, :], kpeR[0:DR, :])
        ag(agin[:], agkv[:])
        bv_b = rawp.tile([1, H * DV], BF16, tag="kpeR")
        nc.sync.dma_start(bv_b[:], wbv_d[0:1, :])
        nc.sync.dma_start(swbv[0:1, :], bv_b[:])
        ag(swbv[:], gwbv[:])
        qb_b = rawp.tile([1, QLT * DQK], BF16, tag="kpeR")
        nc.sync.dma_start(qb_b[:], wqb_d[0:1, :])
        nc.sync.dma_start(swqb[0:1, :], qb_b[:])
        ag(swqb[:], gwqb[:])
        ph.close()

        # =========== phase D: V (own tokens, all heads) -> AG ===========
        ph = ExitStack()
        wv = ph.enter_context(tc.tile_pool(name="wv", bufs=2))
        psD = ph.enter_context(tc.tile_pool(name="psD", bufs=2, space="PSUM"))
        evp = ph.enter_context(tc.tile_pool(name="evp", bufs=3))

        for vc in range(8):             # 8 chunks of 512 v-columns
            wvs = wv.tile([128, KVT, CH], BF16, tag="wvs")
            nc.sync.dma_start(
                wvs[:], gwbv[:, CH * vc:CH * (vc + 1)]
                .rearrange("(l p) c -> p l c", p=128))
            for tt in range(2):         # 2 token tiles of 128
                ps = psD.tile([128, CH], F32, tag="vps")
                for lt in range(KVT):
                    nc.tensor.matmul(
                        ps[:], latkv_n[:, lt, 128 * tt:128 * (tt + 1)],
                        wvs[:, lt, :], start=(lt == 0), stop=(lt == KVT - 1))
                ev = evp.tile([128, CH], BF16, tag="vev")
                nc.scalar.copy(ev[:], ps[:])
                nc.scalar.dma_start(
                    vshard[128 * tt:128 * (tt + 1), CH * vc:CH * (vc + 1)],
                    ev[:])

        # vfull AG issued after ktfull (phase E) so kt is gathered first

        # =========== phase E: K^T (this core's 4 heads, all tokens) -> AG ====
        wkn = ph.enter_context(tc.tile_pool(name="wkn", bufs=1))
        wkns = wkn.tile([128, KVT, 4 * DN], BF16)
        nc.sync.dma_start(wkns[:], wbn_d.rearrange("p (l c) -> p l c", l=KVT))

        agp = ph.enter_context(tc.tile_pool(name="agp", bufs=2))
        for r in range(NCORES):
            slab = agp.tile([128, KVT, OWN], BF16, tag="agslab")
            nc.sync.dma_start(
                slab[:], agkv[(KVL + DR) * r:(KVL + DR) * r + KVL, :]
                .rearrange("(l p) t -> p l t", p=128))
            for hl in range(4):
                ps = psD.tile([128, OWN], F32, tag="ktps")
                for lt in range(KVT):
                    nc.tensor.matmul(ps[:], wkns[:, lt, DN * hl:DN * (hl + 1)],
                                     slab[:, lt, :],
                                     start=(lt == 0), stop=(lt == KVT - 1))
                ev = evp.tile([128, OWN], BF16, tag="ktev")
                nc.scalar.copy(ev[:], ps[:])
                # token-ordered columns: chunk r covers blocks r and 15-r
                nc.scalar.dma_start(
                    ktshard[DN * hl:DN * (hl + 1), 128 * r:128 * (r + 1)],
                    ev[:, 0:128])
                nc.scalar.dma_start(
                    ktshard[DN * hl:DN * (hl + 1),
                            128 * (15 - r):128 * (16 - r)],
                    ev[:, 128:256])

        ag(ktshard[:], ktfull[:])
        vb = agp.tile([1, 128], BF16, tag="agslab")
        nc.sync.dma_start(vb[:], vshard[0:1, 0:128])
        nc.sync.dma_start(vshard[0:1, 0:128], vb[:])
        ag(vshard[:], vfull[:])
        wob = agp.tile([1, H * DV], BF16, tag="agslab")
        nc.sync.dma_start(wob[:], wo_d[0:1, :])
        nc.sync.dma_start(swo[0:1, :], wob[:])
        ag(swo[:], gwo[:])

        # k_pe^T assembly (token-ordered, shared across heads)
        kpeT = const.tile([128, T], BF16)
        for b in range(16):
            rb = min(b, 15 - b)
            colsl = slice(0, 128) if b < 8 else slice(128, 256)
            src_ap = agkv[(KVL + DR) * rb + KVL:(KVL + DR) * rb + KVL + DR,
                          colsl]
            nc.sync.dma_start(kpeT[0:DR, 128 * b:128 * (b + 1)], src_ap)
            nc.sync.dma_start(kpeT[64:64 + DR, 128 * b:128 * (b + 1)], src_ap)
        ph.close()

        # =========== phase F: Q up-projection + rope (all heads) ===========
        qp_pool = st.enter_context(tc.tile_pool(name="qp", bufs=1))
        qTn = qp_pool.tile([128, H, OWN], BF16)
        qTp = qp_pool.tile([128, H // 2, OWN], BF16)
        attnB = qp_pool.tile([128, H, OWN], BF16)

        ph = ExitStack()
        wqb = ph.enter_context(tc.tile_pool(name="wqb", bufs=2))
        psF = ph.enter_context(tc.tile_pool(name="psF", bufs=3, space="PSUM"))
        rp = ph.enter_context(tc.tile_pool(name="rp", bufs=3))

        for h in range(H):
            ws = wqb.tile([128, QLT, DQK], BF16, tag="wqbs")
            nc.sync.dma_start(
                ws[:], gwqb[128 * h:128 * (h + 1), :]
                .rearrange("p (l c) -> p l c", l=QLT))
            pb = 0 if h < 16 else 64
            hs_ = h % 16
            psn = psF.tile([128, OWN], F32, tag="qnps")
            psp = psF.tile([128, OWN], F32, tag="qpps")
            for lt in range(QLT):
                nc.tensor.matmul(psn[:], ws[:, lt, 0:DN], latq_n[:, lt, :],
                                 start=(lt == 0), stop=(lt == QLT - 1))
            for lt in range(QLT):
                nc.tensor.matmul(psp[0:DR, :], ws[:, lt, DN:DQK],
                                 latq_n[:, lt, :],
                                 start=(lt == 0), stop=(lt == QLT - 1))
            nc.scalar.copy(qTn[:, h, :], psn[:])
            praw = rp.tile([128, OWN], F32, tag="praw")
            nc.scalar.copy(praw[0:DR, :], psp[0:DR, :])
            psw = rp.tile([128, OWN], F32, tag="psw")
            nc.sync.dma_start(psw[0:32, :], praw[32:DR, :])
            nc.sync.dma_start(psw[32:DR, :], praw[0:32, :])
            pc = rp.tile([128, OWN], F32, tag="pc")
            nc.vector.tensor_tensor(out=pc[0:DR, :], in0=praw[0:DR, :],
                                    in1=cosT[0:DR, :], op=AL.mult)
            nc.vector.tensor_tensor(out=psw[0:DR, :], in0=psw[0:DR, :],
                                    in1=sinTs[0:DR, :], op=AL.mult)
            if pb == 0:
                nc.vector.tensor_tensor(out=qTp[0:DR, hs_, :], in0=pc[0:DR, :],
                                        in1=psw[0:DR, :], op=AL.add)
            else:
                rshift = rp.tile([128, OWN], BF16, tag="rshift")
                nc.vector.tensor_tensor(out=rshift[0:DR, :], in0=pc[0:DR, :],
                                        in1=psw[0:DR, :], op=AL.add)
                nc.sync.dma_start(qTp[pb:pb + DR, hs_, :], rshift[0:DR, :])
        ph.close()

        # =========== phase G: attention ===========
        ph = ExitStack()
        ap = ph.enter_context(tc.tile_pool(name="ap", bufs=2))
        sp_pool = ph.enter_context(tc.tile_pool(name="spp", bufs=2))
        stt = ph.enter_context(tc.tile_pool(name="stt", bufs=2))
        psG = ph.enter_context(tc.tile_pool(name="psG", bufs=2, space="PSUM"))
        psT = ph.enter_context(tc.tile_pool(name="psT", bufs=4, space="PSUM"))
        psV = ph.enter_context(tc.tile_pool(name="psV", bufs=2, space="PSUM"))

        for h in range(H):
            kt = ap.tile([128, T], BF16, tag="kt")
            nc.sync.dma_start(kt[:], ktfull[DN * h:DN * (h + 1), :])
            vh = ap.tile([128, 16, DV], BF16, tag="vh")
            nc.sync.dma_start(
                vh[:], vfull.rearrange("(k p) c -> p k c", p=128)
                [:, :, DV * h:DV * (h + 1)])

            pb = 0 if h < 16 else 64
            hs_ = h % 16

            # scores per unit: nope+rope MMs, mask (in PSUM), exp -> escr + sum
            Pb = sp_pool.tile([128, 6, CH], BF16, tag="Pb")
            sumu = stt.tile([128, 6], F32, tag="sumu")
            escrs = []
            for u in (0, 1, 2, 5, 3, 4):
                q0 = 128 * UQS[u]
                ps = psG.tile([128, CH], F32, tag="sps", name=f"s{h}_{u}")
                nc.tensor.matmul(ps[:], qTn[:, h, q0:q0 + 128],
                                 kt[:, UKO[u]:UKO[u] + CH],
                                 start=True, stop=False)
                nc.tensor.matmul(ps[:], qTp[pb:pb + DR, hs_, q0:q0 + 128],
                                 kpeT[pb:pb + DR, UKO[u]:UKO[u] + CH],
                                 start=False, stop=True)
                if u in MASKED:
                    mi = MASKED.index(u)
                    nc.vector.tensor_tensor(out=ps[:], in0=ps[:],
                                            in1=mask4[:, mi, :], op=AL.add)
                escr = sp_pool.tile([128, CH], F32, tag="escr", bufs=8,
                                    name=f"e{h}_{u}")
                nc.scalar.activation(escr[:], ps[:], AF.Exp,
                                     bias=ebias[:], scale=SCALING,
                                     accum_out=sumu[:, u:u + 1])
                escrs.append(escr)

            # group sums -> diag(1/sum) matrices
            sB0 = stt.tile([128, 1], F32, tag="sB0")
            nc.vector.tensor_tensor(out=sB0[:], in0=sumu[:, 0:1],
                                    in1=sumu[:, 1:2], op=AL.add)
            sB1 = stt.tile([128, 1], F32, tag="sB1")
            nc.vector.tensor_tensor(out=sB1[:], in0=sumu[:, 2:3],
                                    in1=sumu[:, 5:6], op=AL.add)
            sB = stt.tile([128, 1], F32, tag="sB")
            nc.vector.tensor_tensor(out=sB[:], in0=sB0[:], in1=sB1[:],
                                    op=AL.add)
            sA = stt.tile([128, 1], F32, tag="sA")
            nc.vector.tensor_tensor(out=sA[:], in0=sumu[:, 3:4],
                                    in1=sumu[:, 4:5], op=AL.add)
            rA = stt.tile([128, 1], F32, tag="rA")
            nc.vector.reciprocal(rA[:], sA[:])
            rB = stt.tile([128, 1], F32, tag="rB")
            nc.vector.reciprocal(rB[:], sB[:])

            # normalize during the f32->bf16 cast (q is on partitions here)
            for i, u in enumerate((0, 1, 2, 5, 3, 4)):
                r_ = rA if u in AUNITS else rB
                if i % 3 == 2:
                    nc.scalar.mul(Pb[:, u, :], escrs[i][:], r_[:])
                else:
                    nc.vector.tensor_scalar_mul(Pb[:, u, :], escrs[i][:],
                                                r_[:])

            # P^T + PV accumulated in PSUM
            psVB = psV.tile([128, DV], F32, tag="pv", name=f"pvB{h}")
            psVA = psV.tile([128, DV], F32, tag="pv", name=f"pvA{h}")
            nmm = 0
            for u in range(6):
                isA = u in AUNITS
                dst = psVA if isA else psVB
                first = (u == 3 and True) if isA else (u == 0)
                for kb in range(4):
                    tp = psT.tile([128, 128], BF16, tag="tp")
                    nc.tensor.transpose(tp[:], Pb[:, u, 128 * kb:128 * (kb + 1)],
                                        ident[:])
                    ptT = stt.tile([128, 128], BF16, tag="ptT", bufs=4)
                    if nmm % 3 == 2:
                        nc.scalar.copy(ptT[:], tp[:])
                    else:
                        nc.vector.tensor_copy(ptT[:], tp[:])
                    nmm += 1
                    nc.tensor.matmul(
                        dst[:], vh[:, U_V[u][kb], :], ptT[:],
                        start=(first and kb == 0),
                        stop=((u == 4 and kb == 3) if isA
                              else (u == 5 and kb == 3)))
            nc.scalar.copy(attnB[:, h, 0:128], psVA[:])
            nc.vector.tensor_copy(attnB[:, h, 128:256], psVB[:])
        ph.close()

        # =========== phase H: out projection ===========
        ph = ExitStack()
        wop = ph.enter_context(tc.tile_pool(name="wop", bufs=2))
        psH = ph.enter_context(tc.tile_pool(name="psH", bufs=4, space="PSUM"))
        oev = ph.enter_context(tc.tile_pool(name="oev", bufs=3))
        for oc in range(HID // 128):
            ws = wop.tile([128, H, 128], BF16, tag="wos")
            nc.sync.dma_start(
                ws[:], gwo[128 * oc:128 * (oc + 1), :]
                .rearrange("p (t c) -> p t c", t=H))
            ps = psH.tile([128, OWN], F32, tag="ops")
            for ct in range(H):
                nc.tensor.matmul(ps[:], ws[:, ct, :], attnB[:, ct, :],
                                 start=(ct == 0), stop=(ct == H - 1))
            ev = oev.tile([128, OWN], F32, tag="oev")
            if oc % 2 == 0:
                nc.scalar.copy(ev[:], ps[:])
            else:
                nc.vector.tensor_copy(ev[:], ps[:])
            nc.sync.dma_start(outT_d[128 * oc:128 * (oc + 1), :], ev[:])
        ph.close()
        st.close()

    nc.finalize()
    legalize_sync_waits(nc)
    return nc


_DEINT = np.array([2 * r if r < 32 else 2 * r - 63 for r in range(DR)])


def _host_prep(inputs):
    f32 = np.float32
    bf16 = ml_dtypes.bfloat16
    hs = np.asarray(inputs["hidden_states"], f32)
    cos = np.asarray(inputs["cos"], f32).reshape(T, DR)
    sin = np.asarray(inputs["sin"], f32).reshape(T, DR)
    wq_a = np.asarray(inputs["wq_a"], f32)
    q_ln = np.asarray(inputs["q_a_ln_w"], f32)
    wq_b = np.asarray(inputs["wq_b"], f32)
    wkv_a = np.asarray(inputs["wkv_a"], f32)
    kv_ln = np.asarray(inputs["kv_a_ln_w"], f32)
    wkv_b = np.asarray(inputs["wkv_b"], f32)
    wo = np.asarray(inputs["wo"], f32)

    # fold ln weights into up-projections
    wq_b = wq_b * q_ln[:, None]
    wkv_b = wkv_b * kv_ln[:, None]

    # deinterleave fold: q_pe columns of wq_b, k_pe columns of wkv_a
    wqbp = wq_b.copy()
    for h in range(H):
        pe = wq_b[:, h * DQK + DN:h * DQK + DQK]
        wqbp[:, h * DQK + DN:h * DQK + DQK] = pe[:, _DEINT]
    wkvap = np.zeros((HID, KVT5 * 128), f32)
    wkvap[:, :KVL] = wkv_a[:, :KVL]
    wkvap[:, KVL:KVL + DR] = wkv_a[:, KVL:][:, _DEINT]

    # split wkv_b into nope / v column groups (head-major)
    wkvbn = np.concatenate(
        [wkv_b[:, h * 256:h * 256 + DN] for h in range(H)], axis=1)
    wkvbv = np.concatenate(
        [wkv_b[:, h * 256 + DN:h * 256 + 256] for h in range(H)], axis=1)

    # swizzles: slab-major, partition-contiguous layouts
    WQA = wq_a.reshape(HT, 128, QLT, 128).transpose(2, 1, 0, 3) \
        .reshape(QL, HID).astype(bf16)
    WKVA = wkvap.reshape(HT, 128, KVT5, 128).transpose(2, 1, 0, 3) \
        .reshape(KVT5 * 128, HID).astype(bf16)
    WQB = wqbp.reshape(QLT, 128, H, DQK).transpose(2, 1, 0, 3) \
        .reshape(H * 128, QLT * DQK).astype(bf16)
    WBV = wkvbv.astype(bf16)                      # [512, 4096] natural
    WO = wo.reshape(H, 128, HT, 128).transpose(2, 1, 0, 3) \
        .reshape(HID, H * DV).astype(bf16)

    cosT = np.ascontiguousarray(cos.T)
    sinT = np.ascontiguousarray(sin.T)
    sinTs = sinT.copy()
    sinTs[0:32] = -sinT[0:32]

    ident = np.eye(128, dtype=bf16)
    ones128 = np.ones((128, 1), f32)
    onesrow = np.ones((1, 128), f32)

    def shard(a):
        n = a.shape[0] // NCORES
        return [np.ascontiguousarray(a[c * n:(c + 1) * n]) for c in
                range(NCORES)]

    WQA_s, WKVA_s, WQB_s, WBV_s, WO_s = (shard(WQA), shard(WKVA), shard(WQB),
                                         shard(WBV), shard(WO))

    qr = np.arange(128)[:, None]
    kr = np.arange(CH)[None, :]

    in_maps = []
    for c in range(NCORES):
        bA, bB = c, 15 - c
        own = np.r_[np.arange(128 * bA, 128 * bA + 128),
                    np.arange(128 * bB, 128 * bB + 128)]
        # masks for units u2,u3,u4,u5
        mask4 = np.zeros((128, 4, CH), f32)
        specs = [(bB, 2 * CH, True),          # u2
                 (bA, 0, True),               # u3
                 (bA, CH, c >= 4),            # u4
                 (bB, 3 * CH, c < 4)]         # u5
        for mi, (qb, koff, active) in enumerate(specs):
            if not active:
                mask4[:, mi, :] = NEG
            else:
                qtok = 128 * qb + qr
                ktok = koff + kr
                mask4[:, mi, :] = np.where(ktok <= qtok, 0.0, NEG)

        hid_own = np.ascontiguousarray(hs[own].T)      # [5120, 256]
        hidp = hid_own.reshape(HT, 128, OWN).transpose(1, 0, 2) \
            .reshape(128, HT * OWN).astype(bf16)

        wbn_c = wkvbn[:, 4 * DN * c:4 * DN * (c + 1)]  # [512, 512]
        wbn = wbn_c.reshape(KVT, 128, 4 * DN).transpose(1, 0, 2) \
            .reshape(128, KVT * 4 * DN).astype(bf16)

        in_maps.append({
            "hidp": hidp,
            "cosT": np.ascontiguousarray(cosT[:, own]),
            "sinTs": np.ascontiguousarray(sinTs[:, own]),
            "wqa_sh": WQA_s[c],
            "wkva_sh": WKVA_s[c],
            "wqb_sh": WQB_s[c],
            "wbv_sh": WBV_s[c],
            "wo_sh": WO_s[c],
            "wbn": wbn,
            "mask4": mask4,
            "ident": ident, "ones128": ones128, "onesrow": onesrow,
        })
    return in_maps


_NC_CACHE = None


def _get_nc():
    global _NC_CACHE
    if _NC_CACHE is None:
        _NC_CACHE = build_bass()
    return _NC_CACHE


def run(inputs, trace=False):
    nc = _get_nc()
    in_maps = _host_prep(inputs)
    res = run_bass_kernel_spmd(nc, in_maps, list(range(NCORES)), trace=trace)
    out = np.empty((T, HID), np.float32)
    for c in range(NCORES):
        oT = res.results[c]["outT"]
        out[128 * c:128 * (c + 1)] = oT[:, 0:128].T
        out[128 * (15 - c):128 * (16 - c)] = oT[:, 128:256].T
    return out, res


def kernel(**inputs):
    out, _ = run(inputs, trace=False)
    return out
